# revision 27
# baseline (speedup 1.0000x reference)
"""ComplexLayerNorm Trainium2 kernel (8 NeuronCores, SPMD, F-sharded).

Math (see reference): per-feature 2x2 covariance whitening of (re, im) over
all B*C samples (centered with the batch-only mean mu_b), after subtracting
the complex mean over F, plus complex affine.

v3 restructure (vs the C-sharded v2 baseline):
  * F-sharding: each core owns 256 features (2 chunks of 128 on partitions)
    and ALL B*C = 8192 samples.  The (F,2,2) covariance reduction is then
    fully core-local (no collective on the whitening path!).  Only the
    per-sample complex mean over F needs cross-core combination.
  * The mean partials ride ONE fp32 AllGather ([2,8192] -> [16,8192]) that
    overlaps the apply phase; the 8-way shard sum is folded into the K=17
    correction matmul (16 gathered mean rows + a ones row for beta), so no
    elementwise reduction of the gathered shards is ever done.
  * Mean centering is folded into the PE apply as that K=17 matmul
    (rows: -A[f] @ m plus beta), deleting the in-place centering pass.
  * fp16 everywhere on the streaming path: halves DMA and enables the DVE
    4x perf mode (TensorScalarPtr) for the second-moment/cross stats and
    the batch-sum trees.
  * Two-pass apply: pass 1 (AllGather-independent) does the diagonal-W
    x-matmuls into PSUM and stages uncorrected A@x in SBUF; pass 2 adds the
    K=17 correction via PSUM + one fused DVE/Pool add, then stores.
"""

import numpy as np

import bass_rust
import concourse.bass as bass
import concourse.mybir as mybir
from concourse import tile
from concourse.bass_utils import run_bass_kernel_spmd


def split_multi_waits(nc):
    """The walrus build in this container allows only ONE sync-wait command
    per instruction; Tile emits several.  Split extras into preceding
    single-wait NoOps on the same engine (sequential waits == AND)."""
    cnt = 0
    for bb in nc.main_func.blocks:
        il = bb.instructions
        newlist = []
        changed = False
        for inst in list(il):
            si = inst.sync_info
            waits = list(si.on_wait) if si else []
            if len(waits) > 1:
                changed = True
                for w in waits[:-1]:
                    cnt += 1
                    nop = bass_rust.InstNoOp(name=f"I-wsplit-{cnt}")
                    nop.engine = inst.engine
                    nop.sync_info = mybir.SyncInfo(on_wait=[w], on_update=[])
                    newlist.append(nop)
                inst.sync_info = mybir.SyncInfo(
                    on_wait=[waits[-1]], on_update=list(si.on_update))
            newlist.append(inst)
        if changed:
            il[:] = newlist
    return cnt

FP = mybir.dt.float32
FR = mybir.dt.float32r
F16 = mybir.dt.float16
AF = mybir.ActivationFunctionType
OP = mybir.AluOpType
AX = mybir.AxisListType

B, C, F = 64, 128, 2048
NCORES = 8
FSH = F // NCORES           # 256 features per core
NCH = FSH // 128            # 2 f-chunks of 128 (on partitions)
BC = B * C                  # 8192 samples per core (full batch)
NBB = 4                     # bigblocks of 2048 samples for DMA streaming
BBS = BC // NBB             # 2048
NG = BC // 128              # 64 apply groups (128 samples each)
EPS = 1e-4
NM1 = float(B * C - 1)      # 8191


def build_bass():
    nc = bass.Bass()

    # x, fp16, f-on-partitions: xt[p, 16384*comp + 8192*cc + j]
    #   = x_comp[sample j, f_local = 128*cc + p]
    xt = nc.dram_tensor("xt", [128, 2 * NCH * BC], F16, kind="ExternalInput")
    ident = nc.dram_tensor("ident", [128, 128], FP, kind="ExternalInput")
    onesF = nc.dram_tensor("onesF", [128, 1], F16, kind="ExternalInput")
    # even/odd row selectors for the Arow build: col r all-ones iff r even/odd
    onesEv = nc.dram_tensor("onesEv", [128, 16], F16, kind="ExternalInput")
    onesOd = nc.dram_tensor("onesOd", [128, 16], F16, kind="ExternalInput")
    # gamma for this core's shard, f-on-partitions: [128, NCH]
    g_r = nc.dram_tensor("g_r", [128, NCH], FP, kind="ExternalInput")
    g_i = nc.dram_tensor("g_i", [128, NCH], FP, kind="ExternalInput")
    # beta for this shard, apply-column order: [0, 256*cc + 2*g + c]
    beta_row = nc.dram_tensor("beta_row", [1, 2 * FSH], FR, kind="ExternalInput")
    onesrow = nc.dram_tensor("onesrow", [1, BC], FR, kind="ExternalInput")

    out = nc.dram_tensor("out", [BC, 2 * FSH], F16, kind="ExternalOutput")

    with tile.TileContext(nc) as tc:
        with (
            tc.tile_pool(name="big", bufs=1) as big,
            tc.tile_pool(name="small", bufs=1) as small,
            tc.tile_pool(name="stg", bufs=1) as stgp,
            tc.tile_pool(name="dram", bufs=1, space="DRAM") as dram,
        ):
            # ---- constants to SBUF
            ident_t = small.tile([128, 128], FP, tag="ident")
            nc.sync.dma_start(ident_t[:], ident[:])
            onesF_t = small.tile([128, 1], F16, tag="onesF")
            nc.sync.dma_start(onesF_t[:], onesF[:])
            onesEv_t = small.tile([128, 16], F16, tag="onesEv")
            nc.sync.dma_start(onesEv_t[:], onesEv[:])
            onesOd_t = small.tile([128, 16], F16, tag="onesOd")
            nc.sync.dma_start(onesOd_t[:], onesOd[:])
            g_r_t = small.tile([128, NCH], FP, tag="g_r")
            nc.sync.dma_start(g_r_t[:], g_r[:])
            g_i_t = small.tile([128, NCH], FP, tag="g_i")
            nc.sync.dma_start(g_i_t[:], g_i[:])

            # mg: 16 gathered mean-partial rows + ones row (for beta)
            mg = small.tile([17, BC], FR, tag="mg")
            nc.sync.dma_start(mg[16:17, :], onesrow[:])
            # Arow17: correction-matmul rhs; row 16 = beta
            Arow = small.tile([17, 2 * FSH], FR, tag="Arow")
            nc.sync.dma_start(Arow[16:17, :], beta_row[:])

            # ---- persistent x (fp16, f-on-partitions)
            xT = big.tile([128, 2 * NCH * BC], F16, tag="xT")

            # batch-sums over b: T[comp][p, 128*cc + c]
            T_sb = small.tile([128, 2 * NCH * C], F16, tag="T_sb")
            # second-moment accumulator columns:
            # (m,cc) groups x (4 bigblocks x 2 sub-blocks)
            S_acc = small.tile([128, 48], FP, tag="S_acc")


            from contextlib import ExitStack
            _stk = ExitStack()
            scr = _stk.enter_context(tc.tile_pool(name="scr", bufs=2))
            trp = _stk.enter_context(tc.tile_pool(name="trp", bufs=2))
            msp = _stk.enter_context(tc.tile_pool(name="msp", bufs=2))
            ps_mean = _stk.enter_context(
                tc.tile_pool(name="ps_mean", bufs=1, space="PSUM"))
            ar_in = dram.tile([2, BC], FR, tag="ar_in")
            ar_out = dram.tile([16, BC], FR, tag="ar_out")

            def xsl(comp, cc, lo, n):
                return xT[:, 16384 * comp + BC * cc + lo:
                          16384 * comp + BC * cc + lo + n]

            def xdr(comp, cc, lo, n):
                return xt[:, 16384 * comp + BC * cc + lo:
                          16384 * comp + BC * cc + lo + n]

            # idx(m, cc, b, sub) column in S_acc; m: 0=rr 1=ii 2=ri
            def sidx(m, cc, b, sub):
                return ((m * 2 + cc) * 4 + b) * 2 + sub

            # ---- Phase A: stream bigblocks; stats overlap the DMA
            psm = {}
            for b in range(NBB):
                lo = BBS * b
                for cc in range(NCH):
                    for comp in range(2):
                        nc.sync.dma_start(xsl(comp, cc, lo, BBS),
                                          xdr(comp, cc, lo, BBS))
                    for sub in range(2):
                        xr = xsl(0, cc, lo + 1024 * sub, 1024)
                        xi = xsl(1, cc, lo + 1024 * sub, 1024)
                        # S_rr on ACT (Square w/ accumulate)
                        sa = scr.tile([128, 1024], F16, tag="sq_act")
                        nc.scalar.activation(
                            sa[:], xr, AF.Square,
                            accum_out=S_acc[:, sidx(0, cc, b, sub):
                                            sidx(0, cc, b, sub) + 1])
                        # S_ii and S_ri on DVE (4x fp16 TensorScalarPtr)
                        sd = scr.tile([128, 1024], F16, tag="sq_dve")
                        nc.vector.scalar_tensor_tensor(
                            out=sd[:], in0=xi, scalar=1.0, in1=xi,
                            op0=OP.mult, op1=OP.mult,
                            accum_out=S_acc[:, sidx(1, cc, b, sub):
                                            sidx(1, cc, b, sub) + 1])
                        sx = scr.tile([128, 1024], F16, tag="sq_x")
                        nc.vector.scalar_tensor_tensor(
                            out=sx[:], in0=xr, scalar=1.0, in1=xi,
                            op0=OP.mult, op1=OP.mult,
                            accum_out=S_acc[:, sidx(2, cc, b, sub):
                                            sidx(2, cc, b, sub) + 1])
                    # T batch-sums: Pool fold1, DVE folds 2..4 + accumulate
                    for comp in range(2):
                        xb = xsl(comp, cc, lo, BBS)
                        t1 = trp.tile([128, BBS // 2], F16, tag="t1")
                        nc.gpsimd.tensor_tensor(
                            out=t1[:], in0=xsl(comp, cc, lo, BBS // 2),
                            in1=xsl(comp, cc, lo + BBS // 2, BBS // 2),
                            op=OP.add)
                        t2 = trp.tile([128, BBS // 4], F16, tag="t2")
                        nc.vector.scalar_tensor_tensor(
                            out=t2[:], in0=t1[:, 0:BBS // 4], scalar=1.0,
                            in1=t1[:, BBS // 4:BBS // 2],
                            op0=OP.mult, op1=OP.add)
                        t3 = trp.tile([128, BBS // 8], F16, tag="t3")
                        nc.vector.scalar_tensor_tensor(
                            out=t3[:], in0=t2[:, 0:BBS // 8], scalar=1.0,
                            in1=t2[:, BBS // 8:BBS // 4],
                            op0=OP.mult, op1=OP.add)
                        tdst = T_sb[:, C * (2 * cc + comp):
                                    C * (2 * cc + comp) + C]
                        if b == 0:
                            nc.vector.scalar_tensor_tensor(
                                out=tdst, in0=t3[:, 0:C], scalar=1.0,
                                in1=t3[:, C:2 * C], op0=OP.mult, op1=OP.add)
                        else:
                            t4 = trp.tile([128, C], F16, tag="t4")
                            nc.vector.scalar_tensor_tensor(
                                out=t4[:], in0=t3[:, 0:C], scalar=1.0,
                                in1=t3[:, C:2 * C], op0=OP.mult, op1=OP.add)
                            nc.vector.scalar_tensor_tensor(
                                out=tdst, in0=t4[:], scalar=1.0, in1=tdst,
                                op0=OP.mult, op1=OP.add)
                    # mean-partial matmuls (accumulated over cc in PSUM);
                    # 512-sample block m of half h lands on psum partition
                    # 64*m (legal start partitions), one bank per (comp,h).
                    for comp in range(2):
                        if cc == 0 and comp == 0:
                            psm[b] = [
                                ps_mean.tile([128, 512], FP,
                                             tag=f"psm_{comp2}{h}",
                                             name=f"psm_{comp2}{h}_{b}")
                                for comp2 in range(2) for h in range(2)
                            ]
                        for m in range(4):
                            pt = psm[b][comp * 2 + m // 2]
                            nc.tensor.matmul(
                                pt[64 * (m % 2):64 * (m % 2) + 1, :],
                                onesF_t[:],
                                xsl(comp, cc, lo + 512 * m, 512),
                                start=(cc == 0), stop=(cc == NCH - 1),
                            )
                # move this bigblock's mean partials PSUM -> SBUF (full-tile
                # copies; only rows 0/64 carry data) and drain to DRAM:
                # ar_in[comp, 2048*b + 1024*h + 512*m + j], partition dim
                # strided by 64 picking the two data rows.
                for comp in range(2):
                    mt = msp.tile([128, 1024], FR, tag=f"ms{comp}",
                                  name=f"ms{comp}_{b}")
                    for h in range(2):
                        dst = mt[:, 512 * h:512 * (h + 1)]
                        if comp == 0:
                            nc.scalar.copy(dst, psm[b][comp * 2 + h][:])
                        else:
                            nc.vector.tensor_copy(dst, psm[b][comp * 2 + h][:])
                    src = mt[:].rearrange(
                        "(m o) (h j) -> m o h j", o=64, h=2)[:, 0:1]
                    dst = ar_in[comp:comp + 1,
                                BBS * b:BBS * (b + 1)].rearrange(
                        "o (h m j) -> m o h j", h=2, m=2)
                    nc.sync.dma_start(dst, src)

            # ---- AllGather the mean partials (overlaps everything below)
            nc.gpsimd.collective_compute(
                "AllGather", OP.bypass,
                replica_groups=[list(range(NCORES))],
                ins=[ar_in.opt()],
                outs=[ar_out.opt()],
            )
            nc.sync.dma_start(mg[0:16, :], ar_out[:])

            # ---- stats finalize: S6 / corr6 / cov6  (cols = m*2 + cc)
            S6 = small.tile([128, 6], FP, tag="S6")
            nc.vector.tensor_reduce(
                S6[:], S_acc[:].rearrange("p (g b) -> p g b", b=8),
                AX.X, OP.add)
            corr6 = small.tile([128, 6], FP, tag="corr6")
            for m, (ca, cb) in enumerate(((0, 0), (1, 1), (0, 1))):
                for cc in range(NCH):
                    pr = scr.tile([128, C], F16, tag="sq_x",
                                  name=f"tt_{m}_{cc}")
                    nc.vector.scalar_tensor_tensor(
                        out=pr[:],
                        in0=T_sb[:, C * (2 * cc + ca):C * (2 * cc + ca) + C],
                        scalar=1.0,
                        in1=T_sb[:, C * (2 * cc + cb):C * (2 * cc + cb) + C],
                        op0=OP.mult, op1=OP.mult,
                        accum_out=corr6[:, m * 2 + cc:m * 2 + cc + 1])
            cov6 = small.tile([128, 6], FP, tag="cov6")
            nc.vector.scalar_tensor_tensor(
                out=cov6[:], in0=corr6[:], scalar=-1.0 / B, in1=S6[:],
                op0=OP.mult, op1=OP.add)
            nc.vector.tensor_scalar(
                out=cov6[:], in0=cov6[:], scalar1=1.0 / NM1, scalar2=None,
                op0=OP.mult)
            _stk.close()  # release scratch + mean PSUM

            # ---- Phase C: closed-form 2x2 inverse sqrt, fold gamma
            def stile(tag):
                return small.tile([128, NCH], FP, tag=tag, name=tag)

            arr, cii, bri = stile("arr"), stile("cii"), stile("bri")
            nc.vector.tensor_scalar(out=arr[:], in0=cov6[:, 0:2],
                                    scalar1=EPS, scalar2=None, op0=OP.add)
            nc.vector.tensor_scalar(out=cii[:], in0=cov6[:, 2:4],
                                    scalar1=EPS, scalar2=None, op0=OP.add)
            nc.vector.tensor_copy(bri[:], cov6[:, 4:6])

            det, tmp = stile("det"), stile("tmp")
            nc.vector.tensor_tensor(out=det[:], in0=arr[:], in1=cii[:],
                                    op=OP.mult)
            nc.vector.tensor_tensor(out=tmp[:], in0=bri[:], in1=bri[:],
                                    op=OP.mult)
            nc.vector.tensor_tensor(out=det[:], in0=det[:], in1=tmp[:],
                                    op=OP.subtract)
            s_t = stile("s_t")
            nc.scalar.activation(s_t[:], det[:], AF.Sqrt)
            tsum = stile("tsum")
            nc.vector.tensor_tensor(out=tsum[:], in0=arr[:], in1=cii[:],
                                    op=OP.add)
            nc.vector.scalar_tensor_tensor(out=tsum[:], in0=s_t[:], scalar=2.0,
                                           in1=tsum[:], op0=OP.mult,
                                           op1=OP.add)
            tval = stile("tval")
            nc.scalar.activation(tval[:], tsum[:], AF.Sqrt)
            den, rden = stile("den"), stile("rden")
            nc.vector.tensor_tensor(out=den[:], in0=s_t[:], in1=tval[:],
                                    op=OP.mult)
            nc.vector.reciprocal(rden[:], den[:])

            w_rr, w_ii, wri = stile("w_rr"), stile("w_ii"), stile("wri")
            nc.vector.tensor_tensor(out=w_rr[:], in0=cii[:], in1=s_t[:],
                                    op=OP.add)
            nc.vector.tensor_tensor(out=w_rr[:], in0=w_rr[:], in1=rden[:],
                                    op=OP.mult)
            nc.vector.tensor_tensor(out=w_ii[:], in0=arr[:], in1=s_t[:],
                                    op=OP.add)
            nc.vector.tensor_tensor(out=w_ii[:], in0=w_ii[:], in1=rden[:],
                                    op=OP.mult)
            nc.vector.tensor_tensor(out=wri[:], in0=bri[:], in1=rden[:],
                                    op=OP.mult)
            nc.vector.tensor_scalar(out=wri[:], in0=wri[:], scalar1=-1.0,
                                    scalar2=None, op0=OP.mult)

            # A = G @ W
            a_rr, a_ri = stile("a_rr"), stile("a_ri")
            a_ir, a_ii = stile("a_ir"), stile("a_ii")
            u, v = stile("u"), stile("v")
            for dst, (wa, wb, sgn) in (
                (a_rr, (w_rr, wri, -1.0)),   # g_r*w_rr - g_i*w_ri
                (a_ri, (wri, w_ii, -1.0)),   # g_r*w_ri - g_i*w_ii
            ):
                nc.vector.tensor_tensor(out=u[:], in0=g_r_t[:], in1=wa[:],
                                        op=OP.mult)
                nc.vector.tensor_tensor(out=v[:], in0=g_i_t[:], in1=wb[:],
                                        op=OP.mult)
                nc.vector.tensor_tensor(out=dst[:], in0=u[:], in1=v[:],
                                        op=OP.subtract)
            for dst, (wa, wb) in (
                (a_ir, (w_rr, wri)),         # g_i*w_rr + g_r*w_ri
                (a_ii, (wri, w_ii)),         # g_i*w_ri + g_r*w_ii
            ):
                nc.vector.tensor_tensor(out=u[:], in0=g_i_t[:], in1=wa[:],
                                        op=OP.mult)
                nc.vector.tensor_tensor(out=v[:], in0=g_r_t[:], in1=wb[:],
                                        op=OP.mult)
                nc.vector.tensor_tensor(out=dst[:], in0=u[:], in1=v[:],
                                        op=OP.add)

            # ---- W tiles (fp16), diagonal per chunk: W[p, 2g+c]
            Ws = []
            for cc in range(NCH):
                W_r = small.tile([128, 256], F16, tag=f"W_r{cc}",
                                 name=f"W_r{cc}")
                W_i = small.tile([128, 256], F16, tag=f"W_i{cc}",
                                 name=f"W_i{cc}")
                for W, (ev, od) in ((W_r, (a_rr, a_ir)), (W_i, (a_ri, a_ii))):
                    Wv = W[:].rearrange("p (g c) -> p g c", c=2)
                    nc.vector.tensor_scalar(
                        out=Wv[:, :, 0], in0=ident_t[:],
                        scalar1=ev[:, cc:cc + 1], scalar2=None, op0=OP.mult)
                    nc.vector.tensor_scalar(
                        out=Wv[:, :, 1], in0=ident_t[:],
                        scalar1=od[:, cc:cc + 1], scalar2=None, op0=OP.mult)
                Ws.append((W_r, W_i))

            # ---- Arow rows 0..15: -A coefs in apply-column order, all 16
            # shard-pair rows at once, via matmuls against the W tiles with
            # even/odd selector columns (row 2s   <- -W_r coefs,
            #                            row 2s+1 <- -W_i coefs).
            from contextlib import ExitStack as _ES2
            _stk2 = _ES2()
            ps_t = _stk2.enter_context(
                tc.tile_pool(name="ps_t", bufs=1, space="PSUM"))
            psA = ps_t.tile([16, 2 * FSH], FP, tag="psA")
            for cc in range(NCH):
                W_r, W_i = Ws[cc]
                nc.tensor.matmul(psA[:, 256 * cc:256 * (cc + 1)],
                                 onesEv_t[:], W_r[:], start=True, stop=False)
                nc.tensor.matmul(psA[:, 256 * cc:256 * (cc + 1)],
                                 onesOd_t[:], W_i[:], start=False, stop=True)
            nc.vector.tensor_scalar(out=Arow[0:16, :], in0=psA[:],
                                    scalar1=-1.0, scalar2=None, op0=OP.mult)

            # ---- Phase D: two-pass apply
            stg = stgp.tile([128, NG * 512], F16, tag="stg")
            _stk3 = _ES2()
            ps_o = _stk3.enter_context(
                tc.tile_pool(name="ps_o", bufs=6, space="PSUM"))
            corrp = _stk3.enter_context(tc.tile_pool(name="corrp", bufs=2))

            # pass 1: diagonal-W matmuls, stage uncorrected A@x
            for g in range(NG):
                po = ps_o.tile([128, 512], FP, tag="po", name=f"po1_{g}")
                for cc in range(NCH):
                    W_r, W_i = Ws[cc]
                    nc.tensor.matmul(
                        po[:, 256 * cc:256 * (cc + 1)],
                        xsl(0, cc, 128 * g, 128), W_r[:],
                        start=True, stop=False)
                    nc.tensor.matmul(
                        po[:, 256 * cc:256 * (cc + 1)],
                        xsl(1, cc, 128 * g, 128), W_i[:],
                        start=False, stop=True)
                dst = stg[:, 512 * g:512 * (g + 1)]
                if g % 2 == 0:
                    nc.vector.tensor_copy(dst, po[:])
                else:
                    nc.scalar.copy(dst, po[:])

            # pass 2: K=17 correction (means x -A + beta), add, store.
            # GPSIMD can't read PSUM: even groups add directly on DVE;
            # odd groups stage via ACT copy, then add fp16-only on Pool.
            for g in range(NG):
                po = ps_o.tile([128, 512], FP, tag="po", name=f"po2_{g}")
                nc.tensor.matmul(
                    po[:],
                    mg[:, 128 * g:128 * (g + 1)],
                    Arow[:],
                    start=True, stop=True)
                dst = stg[:, 512 * g:512 * (g + 1)]
                if g % 2 == 0:
                    nc.vector.scalar_tensor_tensor(
                        out=dst, in0=dst, scalar=1.0, in1=po[:],
                        op0=OP.mult, op1=OP.add)
                else:
                    ct = corrp.tile([128, 512], F16, tag="ct")
                    nc.scalar.copy(ct[:], po[:])
                    nc.gpsimd.tensor_tensor(
                        out=dst, in0=dst, in1=ct[:], op=OP.add)
                if g % 4 == 3:
                    g0 = g - 3
                    dstd = out.rearrange("(a p) f -> p a f", p=128)[
                        :, g0:g0 + 4, :]
                    src = stg[:, 512 * g0:512 * (g + 1)].rearrange(
                        "p (a q) -> p a q", q=512)
                    if (g // 4) % 2 == 0:
                        nc.sync.dma_start(dstd, src)
                    else:
                        nc.scalar.dma_start(dstd, src)
            _stk3.close()
            _stk2.close()

    split_multi_waits(nc)
    return nc


_CACHE = {}


def _get_nc():
    if "nc" not in _CACHE:
        _CACHE["nc"] = build_bass()
    return _CACHE["nc"]


def _constants():
    if "consts" not in _CACHE:
        _CACHE["consts"] = {
            "ident": np.eye(128, dtype=np.float32),
            "onesF": np.full((128, 1), 1.0 / F, dtype=np.float16),
            "onesrow": np.ones((1, BC), dtype=np.float32),
            "onesEv": np.tile((np.arange(16) % 2 == 0).astype(np.float16),
                              (128, 1)),
            "onesOd": np.tile((np.arange(16) % 2 == 1).astype(np.float16),
                              (128, 1)),
        }
    return _CACHE["consts"]


def _host_xt(xr, xi, fsl):
    """Build xt[p, 16384*comp + 8192*cc + j] = x_comp[j, 128*cc + p]."""
    halves = []
    for x in (xr, xi):
        xs = x[:, fsl].reshape(BC, NCH, 128)        # (j, cc, p)
        halves.append(np.transpose(xs, (2, 1, 0)).reshape(128, NCH * BC))
    return np.ascontiguousarray(
        np.concatenate(halves, axis=1)).astype(np.float16)


def kernel(x_real, x_imag, gamma_r, gamma_i, beta_r, beta_i):
    x_real = np.asarray(x_real, dtype=np.float32).reshape(BC, F)
    x_imag = np.asarray(x_imag, dtype=np.float32).reshape(BC, F)
    gamma_r = np.asarray(gamma_r, dtype=np.float32)
    gamma_i = np.asarray(gamma_i, dtype=np.float32)
    beta_r = np.asarray(beta_r, dtype=np.float32)
    beta_i = np.asarray(beta_i, dtype=np.float32)

    nc = _get_nc()
    consts = _constants()

    in_maps = []
    for k in range(NCORES):
        fsl = slice(FSH * k, FSH * (k + 1))
        g_r_t = np.ascontiguousarray(gamma_r[fsl].reshape(NCH, 128).T)
        g_i_t = np.ascontiguousarray(gamma_i[fsl].reshape(NCH, 128).T)
        beta_row = np.ascontiguousarray(
            np.stack([beta_r[fsl], beta_i[fsl]], axis=-1).reshape(1, 2 * FSH))
        in_maps.append({
            "xt": _host_xt(x_real, x_imag, fsl),
            "g_r": g_r_t, "g_i": g_i_t, "beta_row": beta_row,
            **consts,
        })

    res = run_bass_kernel_spmd(nc, in_maps, list(range(NCORES)))

    full = np.empty((B, C, F, 2), dtype=np.float32)
    for k in range(NCORES):
        full[:, :, FSH * k:FSH * (k + 1)] = (
            np.asarray(res.results[k]["out"]).astype(np.float32)
            .reshape(B, C, FSH, 2)
        )
    return full


# revision 29
# speedup vs baseline: 1.4178x; 1.4178x over previous
"""ComplexLayerNorm Trainium2 kernel (8 NeuronCores, SPMD, F-sharded).

Math (see reference): per-feature 2x2 covariance whitening of (re, im) over
all B*C samples (centered with the batch-only mean mu_b), after subtracting
the complex mean over F, plus complex affine.

v4 design:
  * F-sharding: each core owns 256 features (2 chunks of 128 on partitions)
    and ALL B*C = 8192 samples; the covariance path is fully core-local.
  * Only the per-sample complex mean over F crosses cores: partials are
    scaled x64, quantized to fp8e4 (plenty for a mean-subtraction term) and
    ride ONE AllGather that overlaps the apply phase.  The 8-way shard sum
    AND the beta add are folded into a K=17 correction matmul (16 gathered
    mean rows + a ones row), whose rhs carries -A/64 coefficients.
  * Engine assignment respects the cost model: ACT does Square+accum and
    the PSUM mean-staging copies; Pool does the cross products and the
    first T-tree fold (TensorTensor); DVE does the x_i^2 products (2x
    fp16 TT), all second-moment accumulations (4x fp16 TensorScalarPtr
    accum), and the deep T-tree folds.
  * Two-pass apply: pass 1 (collective-independent) does the diagonal-W
    x-matmuls into PSUM and stages uncorrected A@x; pass 2 adds the K=17
    correction (DVE stt from PSUM on even groups, ACT-copy + Pool add on
    odd groups) and stores.  Emission interleaves the passes so the PE
    never idles waiting for the collective.
"""

import numpy as np
import ml_dtypes

import bass_rust
import concourse.bass as bass
import concourse.mybir as mybir
from concourse import tile
from concourse.bass_utils import run_bass_kernel_spmd


def split_multi_waits(nc):
    """The walrus build in this container allows only ONE sync-wait command
    per instruction; Tile emits several.  Split extras into preceding
    single-wait NoOps on the same engine (sequential waits == AND)."""
    cnt = 0
    for bb in nc.main_func.blocks:
        il = bb.instructions
        newlist = []
        changed = False
        for inst in list(il):
            si = inst.sync_info
            waits = list(si.on_wait) if si else []
            if len(waits) > 1:
                changed = True
                for w in waits[:-1]:
                    cnt += 1
                    nop = bass_rust.InstNoOp(name=f"I-wsplit-{cnt}")
                    nop.engine = inst.engine
                    nop.sync_info = mybir.SyncInfo(on_wait=[w], on_update=[])
                    newlist.append(nop)
                inst.sync_info = mybir.SyncInfo(
                    on_wait=[waits[-1]], on_update=list(si.on_update))
            newlist.append(inst)
        if changed:
            il[:] = newlist
    return cnt

FP = mybir.dt.float32
FR = mybir.dt.float32r
F16 = mybir.dt.float16
F8 = mybir.dt.float8e4
AF = mybir.ActivationFunctionType
OP = mybir.AluOpType
AX = mybir.AxisListType

B, C, F = 64, 128, 2048
NCORES = 8
FSH = F // NCORES           # 256 features per core
NCH = FSH // 128            # 2 f-chunks of 128 (on partitions)
BC = B * C                  # 8192 samples per core (full batch)
NBB = 4                     # bigblocks of 2048 samples for DMA streaming
BBS = BC // NBB             # 2048
NG = BC // 128              # 64 apply groups (128 samples each)
EPS = 1e-4
NM1 = float(B * C - 1)      # 8191
MSCALE = 64.0               # fp8 mean-partial scaling (folded into consts)


def build_bass():
    nc = bass.Bass()

    # x, fp16, f-on-partitions: xt[p, 16384*comp + 8192*cc + j]
    #   = x_comp[sample j, f_local = 128*cc + p]
    xt = nc.dram_tensor("xt", [128, 2 * NCH * BC], F16, kind="ExternalInput")
    ident = nc.dram_tensor("ident", [128, 128], FP, kind="ExternalInput")
    # mean-matmul weights: MSCALE/F
    onesF = nc.dram_tensor("onesF", [128, 1], F16, kind="ExternalInput")
    # even/odd row selectors for the Arow build
    onesEv = nc.dram_tensor("onesEv", [128, 16], F16, kind="ExternalInput")
    onesOd = nc.dram_tensor("onesOd", [128, 16], F16, kind="ExternalInput")
    # gamma for this core's shard, f-on-partitions: [128, NCH]
    g_r = nc.dram_tensor("g_r", [128, NCH], FP, kind="ExternalInput")
    g_i = nc.dram_tensor("g_i", [128, NCH], FP, kind="ExternalInput")
    # beta for this shard (fp8), apply-column order: [0, 256*cc + 2*g + c]
    beta_row = nc.dram_tensor("beta_row", [1, 2 * FSH], F8,
                              kind="ExternalInput")
    onesrow = nc.dram_tensor("onesrow", [1, BC], F8, kind="ExternalInput")

    out = nc.dram_tensor("out", [BC, 2 * FSH], F16, kind="ExternalOutput")

    with tile.TileContext(nc) as tc:
        with (
            tc.tile_pool(name="big", bufs=1) as big,
            tc.tile_pool(name="small", bufs=1) as small,
            tc.tile_pool(name="stg", bufs=1) as stgp,
            tc.tile_pool(name="dram", bufs=1, space="DRAM") as dram,
        ):
            # ---- constants to SBUF
            ident_t = small.tile([128, 128], FP, tag="ident")
            nc.sync.dma_start(ident_t[:], ident[:])
            onesF_t = small.tile([128, 1], F16, tag="onesF")
            nc.sync.dma_start(onesF_t[:], onesF[:])
            onesEv_t = small.tile([128, 16], F16, tag="onesEv")
            nc.sync.dma_start(onesEv_t[:], onesEv[:])
            onesOd_t = small.tile([128, 16], F16, tag="onesOd")
            nc.sync.dma_start(onesOd_t[:], onesOd[:])
            g_r_t = small.tile([128, NCH], FP, tag="g_r")
            nc.sync.dma_start(g_r_t[:], g_r[:])
            g_i_t = small.tile([128, NCH], FP, tag="g_i")
            nc.sync.dma_start(g_i_t[:], g_i[:])

            # mg: 16 gathered fp8 mean-partial rows + ones row (for beta)
            mg = small.tile([17, BC], F8, tag="mg")
            # Arow17: correction-matmul rhs (fp8); row 16 = beta
            Arow = small.tile([17, 2 * FSH], F8, tag="Arow")

            # ---- persistent x (fp16, f-on-partitions)
            xT = big.tile([128, 2 * NCH * BC], F16, tag="xT")

            # batch-sums over b: T_sb[p, 128*(2*cc+comp) + c]
            T_sb = small.tile([128, 2 * NCH * C], F16, tag="T_sb")
            # second-moment accumulator columns:
            #   m=0 (rr): 8 cols (cc,b); m=1 (ii): 4 cols (cc,h2);
            #   m=2 (ri): 8 cols (cc,b)
            S_acc = small.tile([128, 20], FP, tag="S_acc")

            from contextlib import ExitStack
            _stk = ExitStack()
            scr = _stk.enter_context(tc.tile_pool(name="scr", bufs=1))
            sc2 = _stk.enter_context(tc.tile_pool(name="sc2", bufs=2))
            trp = _stk.enter_context(tc.tile_pool(name="trp", bufs=2))
            msp = _stk.enter_context(tc.tile_pool(name="msp", bufs=2))
            ps_mean = _stk.enter_context(
                tc.tile_pool(name="ps_mean", bufs=2, space="PSUM"))
            ar_in = dram.tile([2, BC], F8, tag="ar_in")
            ar_out = dram.tile([16, BC], F8, tag="ar_out")

            def xsl(comp, cc, lo, n):
                return xT[:, 16384 * comp + BC * cc + lo:
                          16384 * comp + BC * cc + lo + n]

            def xdr(comp, cc, lo, n):
                return xt[:, 16384 * comp + BC * cc + lo:
                          16384 * comp + BC * cc + lo + n]

            # ---- Phase A: stream bigblocks; stats overlap the DMA
            prod_i = {}
            for b in range(NBB):
                lo = BBS * b
                h2, hb = b // 2, b % 2
                for cc in range(NCH):
                    for comp in range(2):
                        nc.sync.dma_start(xsl(comp, cc, lo, BBS),
                                          xdr(comp, cc, lo, BBS))
                    xr = xsl(0, cc, lo, BBS)
                    xi = xsl(1, cc, lo, BBS)
                    # S_rr on ACT: Square w/ accumulate (2048 grain)
                    sa = scr.tile([128, BBS], F16, tag="sq_act")
                    nc.scalar.activation(
                        sa[:], xr, AF.Square,
                        accum_out=S_acc[:, cc * 4 + b:cc * 4 + b + 1])
                    # x_i^2 products on DVE (2x fp16 TT), 4096-grain accum
                    if hb == 0:
                        prod_i[cc] = scr.tile([128, 2 * BBS], F16,
                                              tag=f"prod_i{cc}",
                                              name=f"prod_i{cc}_{h2}")
                    nc.vector.tensor_tensor(
                        out=prod_i[cc][:, BBS * hb:BBS * (hb + 1)],
                        in0=xi, in1=xi, op=OP.mult)
                    if hb == 1:
                        nc.vector.tensor_scalar(
                            out=prod_i[cc][:], in0=prod_i[cc][:],
                            scalar1=1.0, scalar2=0.0, op0=OP.mult,
                            op1=OP.add,
                            accum_out=S_acc[:, 8 + cc * 2 + h2:
                                            8 + cc * 2 + h2 + 1])
                    # cross products on Pool, accum on DVE (4x TSP)
                    px = sc2.tile([128, BBS], F16, tag="prod_x")
                    nc.gpsimd.tensor_tensor(out=px[:], in0=xr, in1=xi,
                                            op=OP.mult)
                    nc.vector.tensor_scalar(
                        out=px[:], in0=px[:], scalar1=1.0, scalar2=0.0,
                        op0=OP.mult, op1=OP.add,
                        accum_out=S_acc[:, 12 + cc * 4 + b:
                                        12 + cc * 4 + b + 1])
                    # mean-partial matmuls: 512-block m of this bigblock
                    # lands on psum partition 64*(m%2), col 512*(m//2).
                    for comp in range(2):
                        if cc == 0 and comp == 0:
                            pm = [ps_mean.tile([128, 1024], FP,
                                               tag=f"psm{c2}",
                                               name=f"psm{c2}_{b}")
                                  for c2 in range(2)]
                            prod_i[("pm", b)] = pm
                        pm = prod_i[("pm", b)]
                        for m in range(4):
                            nc.tensor.matmul(
                                pm[comp][64 * (m % 2):64 * (m % 2) + 1,
                                         512 * (m // 2):512 * (m // 2) + 512],
                                onesF_t[:],
                                xsl(comp, cc, lo + 512 * m, 512),
                                start=(cc == 0), stop=(cc == NCH - 1),
                            )
                # T-tree at 4096-grain: after odd bigblocks
                if hb == 1:
                    lo2 = 2 * BBS * h2
                    for cc in range(NCH):
                        for comp in range(2):
                            xb = xsl(comp, cc, lo2, 2 * BBS)
                            t1 = trp.tile([128, BBS], F16, tag="t1")
                            nc.gpsimd.tensor_tensor(
                                out=t1[:], in0=xsl(comp, cc, lo2, BBS),
                                in1=xsl(comp, cc, lo2 + BBS, BBS), op=OP.add)
                            t2 = trp.tile([128, BBS // 2], F16, tag="t2")
                            nc.vector.tensor_tensor(
                                out=t2[:], in0=t1[:, 0:BBS // 2],
                                in1=t1[:, BBS // 2:BBS], op=OP.add)
                            t3 = trp.tile([128, BBS // 4], F16, tag="t3")
                            nc.vector.tensor_tensor(
                                out=t3[:], in0=t2[:, 0:BBS // 4],
                                in1=t2[:, BBS // 4:BBS // 2], op=OP.add)
                            t4 = trp.tile([128, BBS // 8], F16, tag="t4")
                            nc.vector.tensor_tensor(
                                out=t4[:], in0=t3[:, 0:BBS // 8],
                                in1=t3[:, BBS // 8:BBS // 4], op=OP.add)
                            td = T_sb[:, C * (2 * cc + comp):
                                      C * (2 * cc + comp) + C]
                            if h2 == 0:
                                nc.vector.tensor_tensor(
                                    out=td, in0=t4[:, 0:C], in1=t4[:, C:2 * C],
                                    op=OP.add)
                            else:
                                nc.vector.tensor_tensor(
                                    out=td, in0=td, in1=t4[:, 0:C], op=OP.add)
                                nc.vector.tensor_tensor(
                                    out=td, in0=td, in1=t4[:, C:2 * C],
                                    op=OP.add)
                # stage mean partials PSUM -> SBUF (fp8) and drain to DRAM:
                # ar_in[comp, 2048*b + 1024*h + 512*m' + j] where data rows
                # sit at psum partitions {0,64} (m'), cols 512*h.
                pm = prod_i[("pm", b)]
                for comp in range(2):
                    mt = msp.tile([128, 1024], F8, tag=f"ms{comp}",
                                  name=f"ms{comp}_{b}")
                    nc.scalar.copy(mt[:], pm[comp][:])
                    src = mt[:].rearrange(
                        "(m o) (h j) -> m o h j", o=64, h=2)[:, 0:1]
                    dst = ar_in[comp:comp + 1,
                                BBS * b:BBS * (b + 1)].rearrange(
                        "o (h m j) -> m o h j", h=2, m=2)
                    nc.sync.dma_start(dst, src)

            # ---- AllGather the fp8 mean partials (overlaps everything below)
            nc.gpsimd.collective_compute(
                "AllGather", OP.bypass,
                replica_groups=[list(range(NCORES))],
                ins=[ar_in.opt()],
                outs=[ar_out.opt()],
            )
            # constants for pass 2 (DMA during the collective window)
            nc.scalar.dma_start(mg[16:17, :], onesrow[:])
            nc.scalar.dma_start(Arow[16:17, :], beta_row[:])
            # gather result in two halves so early groups start sooner
            for hf in range(2):
                nc.sync.dma_start(
                    mg[0:16, 4096 * hf:4096 * (hf + 1)],
                    ar_out[:, 4096 * hf:4096 * (hf + 1)])

            # ---- stats finalize: S6 (cols m*2+cc), corr6, cov6
            S6 = small.tile([128, 6], FP, tag="S6")
            nc.vector.tensor_reduce(
                S6[:, 0:2], S_acc[:, 0:8].rearrange("p (g b) -> p g b", b=4),
                AX.X, OP.add)
            nc.vector.tensor_reduce(
                S6[:, 2:4], S_acc[:, 8:12].rearrange("p (g b) -> p g b", b=2),
                AX.X, OP.add)
            nc.vector.tensor_reduce(
                S6[:, 4:6], S_acc[:, 12:20].rearrange("p (g b) -> p g b", b=4),
                AX.X, OP.add)
            corr6 = small.tile([128, 6], FP, tag="corr6")
            for m, (ca, cb) in enumerate(((0, 0), (1, 1), (0, 1))):
                for cc in range(NCH):
                    pr = sc2.tile([128, C], F16, tag="prod_x",
                                  name=f"tt_{m}_{cc}")
                    nc.vector.scalar_tensor_tensor(
                        out=pr[:],
                        in0=T_sb[:, C * (2 * cc + ca):C * (2 * cc + ca) + C],
                        scalar=1.0,
                        in1=T_sb[:, C * (2 * cc + cb):C * (2 * cc + cb) + C],
                        op0=OP.mult, op1=OP.mult,
                        accum_out=corr6[:, m * 2 + cc:m * 2 + cc + 1])
            cov6 = small.tile([128, 6], FP, tag="cov6")
            nc.vector.scalar_tensor_tensor(
                out=cov6[:], in0=corr6[:], scalar=-1.0 / B, in1=S6[:],
                op0=OP.mult, op1=OP.add)
            nc.vector.tensor_scalar(
                out=cov6[:], in0=cov6[:], scalar1=1.0 / NM1, scalar2=None,
                op0=OP.mult)
            _stk.close()  # release scratch + mean PSUM

            # ---- Phase C: closed-form 2x2 inverse sqrt, fold gamma
            def stile(tag):
                return small.tile([128, NCH], FP, tag=tag, name=tag)

            arr, cii, bri = stile("arr"), stile("cii"), stile("bri")
            nc.vector.tensor_scalar(out=arr[:], in0=cov6[:, 0:2],
                                    scalar1=EPS, scalar2=None, op0=OP.add)
            nc.vector.tensor_scalar(out=cii[:], in0=cov6[:, 2:4],
                                    scalar1=EPS, scalar2=None, op0=OP.add)
            nc.vector.tensor_copy(bri[:], cov6[:, 4:6])

            det, tmp = stile("det"), stile("tmp")
            nc.vector.tensor_tensor(out=det[:], in0=arr[:], in1=cii[:],
                                    op=OP.mult)
            nc.vector.tensor_tensor(out=tmp[:], in0=bri[:], in1=bri[:],
                                    op=OP.mult)
            nc.vector.tensor_tensor(out=det[:], in0=det[:], in1=tmp[:],
                                    op=OP.subtract)
            s_t = stile("s_t")
            nc.scalar.activation(s_t[:], det[:], AF.Sqrt)
            tsum = stile("tsum")
            nc.vector.tensor_tensor(out=tsum[:], in0=arr[:], in1=cii[:],
                                    op=OP.add)
            nc.vector.scalar_tensor_tensor(out=tsum[:], in0=s_t[:], scalar=2.0,
                                           in1=tsum[:], op0=OP.mult,
                                           op1=OP.add)
            tval = stile("tval")
            nc.scalar.activation(tval[:], tsum[:], AF.Sqrt)
            den, rden = stile("den"), stile("rden")
            nc.vector.tensor_tensor(out=den[:], in0=s_t[:], in1=tval[:],
                                    op=OP.mult)
            nc.vector.reciprocal(rden[:], den[:])

            w_rr, w_ii, wri = stile("w_rr"), stile("w_ii"), stile("wri")
            nc.vector.tensor_tensor(out=w_rr[:], in0=cii[:], in1=s_t[:],
                                    op=OP.add)
            nc.vector.tensor_tensor(out=w_rr[:], in0=w_rr[:], in1=rden[:],
                                    op=OP.mult)
            nc.vector.tensor_tensor(out=w_ii[:], in0=arr[:], in1=s_t[:],
                                    op=OP.add)
            nc.vector.tensor_tensor(out=w_ii[:], in0=w_ii[:], in1=rden[:],
                                    op=OP.mult)
            nc.vector.tensor_tensor(out=wri[:], in0=bri[:], in1=rden[:],
                                    op=OP.mult)
            nc.vector.tensor_scalar(out=wri[:], in0=wri[:], scalar1=-1.0,
                                    scalar2=None, op0=OP.mult)

            # A = G @ W
            a_rr, a_ri = stile("a_rr"), stile("a_ri")
            a_ir, a_ii = stile("a_ir"), stile("a_ii")
            u, v = stile("u"), stile("v")
            for dst, (wa, wb) in ((a_rr, (w_rr, wri)), (a_ri, (wri, w_ii))):
                nc.vector.tensor_tensor(out=u[:], in0=g_r_t[:], in1=wa[:],
                                        op=OP.mult)
                nc.vector.tensor_tensor(out=v[:], in0=g_i_t[:], in1=wb[:],
                                        op=OP.mult)
                nc.vector.tensor_tensor(out=dst[:], in0=u[:], in1=v[:],
                                        op=OP.subtract)
            for dst, (wa, wb) in ((a_ir, (w_rr, wri)), (a_ii, (wri, w_ii))):
                nc.vector.tensor_tensor(out=u[:], in0=g_i_t[:], in1=wa[:],
                                        op=OP.mult)
                nc.vector.tensor_tensor(out=v[:], in0=g_r_t[:], in1=wb[:],
                                        op=OP.mult)
                nc.vector.tensor_tensor(out=dst[:], in0=u[:], in1=v[:],
                                        op=OP.add)

            # ---- W tiles (fp16), diagonal per chunk: W[p, 2g+c]
            Ws = []
            for cc in range(NCH):
                W_r = small.tile([128, 256], F16, tag=f"W_r{cc}",
                                 name=f"W_r{cc}")
                W_i = small.tile([128, 256], F16, tag=f"W_i{cc}",
                                 name=f"W_i{cc}")
                for W, (ev, od) in ((W_r, (a_rr, a_ir)), (W_i, (a_ri, a_ii))):
                    Wv = W[:].rearrange("p (g c) -> p g c", c=2)
                    nc.vector.tensor_scalar(
                        out=Wv[:, :, 0], in0=ident_t[:],
                        scalar1=ev[:, cc:cc + 1], scalar2=None, op0=OP.mult)
                    nc.vector.tensor_scalar(
                        out=Wv[:, :, 1], in0=ident_t[:],
                        scalar1=od[:, cc:cc + 1], scalar2=None, op0=OP.mult)
                Ws.append((W_r, W_i))

            # ---- Arow rows 0..15: -A/MSCALE coefs in apply-column order,
            # via matmuls against the W tiles with even/odd selectors.
            from contextlib import ExitStack as _ES2
            _stk2 = _ES2()
            ps_t = _stk2.enter_context(
                tc.tile_pool(name="ps_t", bufs=1, space="PSUM"))
            psA = ps_t.tile([16, 2 * FSH], FP, tag="psA")
            for cc in range(NCH):
                W_r, W_i = Ws[cc]
                nc.tensor.matmul(psA[:, 256 * cc:256 * (cc + 1)],
                                 onesEv_t[:], W_r[:], start=True, stop=False)
                nc.tensor.matmul(psA[:, 256 * cc:256 * (cc + 1)],
                                 onesOd_t[:], W_i[:], start=False, stop=True)
            nc.vector.tensor_scalar(out=Arow[0:16, :], in0=psA[:],
                                    scalar1=-1.0 / MSCALE, scalar2=None,
                                    op0=OP.mult)

            # ---- Phase D: two-pass apply with interleaved emission
            stg = stgp.tile([128, NG * 512], F16, tag="stg")
            _stk3 = _ES2()
            ps_o = _stk3.enter_context(
                tc.tile_pool(name="ps_o", bufs=6, space="PSUM"))
            corrp = _stk3.enter_context(tc.tile_pool(name="corrp", bufs=2))

            def pass1(g):
                po = ps_o.tile([128, 512], FP, tag="po", name=f"po1_{g}")
                for cc in range(NCH):
                    W_r, W_i = Ws[cc]
                    nc.tensor.matmul(
                        po[:, 256 * cc:256 * (cc + 1)],
                        xsl(0, cc, 128 * g, 128), W_r[:],
                        start=True, stop=False)
                    nc.tensor.matmul(
                        po[:, 256 * cc:256 * (cc + 1)],
                        xsl(1, cc, 128 * g, 128), W_i[:],
                        start=False, stop=True)
                dst = stg[:, 512 * g:512 * (g + 1)]
                if g % 2 == 0:
                    nc.vector.tensor_copy(dst, po[:])
                else:
                    nc.scalar.copy(dst, po[:])

            def pass2(g):
                po = ps_o.tile([128, 512], FP, tag="po", name=f"po2_{g}")
                nc.tensor.matmul(
                    po[:], mg[:, 128 * g:128 * (g + 1)], Arow[:],
                    start=True, stop=True)
                dst = stg[:, 512 * g:512 * (g + 1)]
                if g % 2 == 0:
                    nc.vector.scalar_tensor_tensor(
                        out=dst, in0=dst, scalar=1.0, in1=po[:],
                        op0=OP.mult, op1=OP.add)
                else:
                    ct = corrp.tile([128, 512], F16, tag="ct")
                    nc.scalar.copy(ct[:], po[:])
                    nc.gpsimd.tensor_tensor(
                        out=dst, in0=dst, in1=ct[:], op=OP.add)
                if g % 4 == 3:
                    g0 = g - 3
                    dstd = out.rearrange("(a p) f -> p a f", p=128)[
                        :, g0:g0 + 4, :]
                    src = stg[:, 512 * g0:512 * (g + 1)].rearrange(
                        "p (a q) -> p a q", q=512)
                    if (g // 4) % 2 == 0:
                        nc.sync.dma_start(dstd, src)
                    else:
                        nc.scalar.dma_start(dstd, src)

            for g in range(40):
                pass1(g)
            p2 = 0
            for g in range(40, NG):
                pass1(g)
                pass2(p2)
                pass2(p2 + 1)
                p2 += 2
            while p2 < NG:
                pass2(p2)
                p2 += 1
            _stk3.close()
            _stk2.close()

    split_multi_waits(nc)
    return nc


_CACHE = {}


def _get_nc():
    if "nc" not in _CACHE:
        _CACHE["nc"] = build_bass()
    return _CACHE["nc"]


def _constants():
    if "consts" not in _CACHE:
        _CACHE["consts"] = {
            "ident": np.eye(128, dtype=np.float32),
            "onesF": np.full((128, 1), MSCALE / F, dtype=np.float16),
            "onesrow": np.ones((1, BC), dtype=ml_dtypes.float8_e4m3),
            "onesEv": np.tile((np.arange(16) % 2 == 0).astype(np.float16),
                              (128, 1)),
            "onesOd": np.tile((np.arange(16) % 2 == 1).astype(np.float16),
                              (128, 1)),
        }
    return _CACHE["consts"]


def _host_xt(xr, xi, fsl):
    """Build xt[p, 16384*comp + 8192*cc + j] = x_comp[j, 128*cc + p]."""
    halves = []
    for x in (xr, xi):
        xs = x[:, fsl].reshape(BC, NCH, 128)        # (j, cc, p)
        halves.append(np.transpose(xs, (2, 1, 0)).reshape(128, NCH * BC))
    return np.ascontiguousarray(
        np.concatenate(halves, axis=1)).astype(np.float16)


def kernel(x_real, x_imag, gamma_r, gamma_i, beta_r, beta_i):
    x_real = np.asarray(x_real, dtype=np.float32).reshape(BC, F)
    x_imag = np.asarray(x_imag, dtype=np.float32).reshape(BC, F)
    gamma_r = np.asarray(gamma_r, dtype=np.float32)
    gamma_i = np.asarray(gamma_i, dtype=np.float32)
    beta_r = np.asarray(beta_r, dtype=np.float32)
    beta_i = np.asarray(beta_i, dtype=np.float32)

    nc = _get_nc()
    consts = _constants()

    in_maps = []
    for k in range(NCORES):
        fsl = slice(FSH * k, FSH * (k + 1))
        g_r_t = np.ascontiguousarray(gamma_r[fsl].reshape(NCH, 128).T)
        g_i_t = np.ascontiguousarray(gamma_i[fsl].reshape(NCH, 128).T)
        beta_row = np.ascontiguousarray(
            np.stack([beta_r[fsl], beta_i[fsl]], axis=-1).reshape(1, 2 * FSH)
        ).astype(ml_dtypes.float8_e4m3)
        in_maps.append({
            "xt": _host_xt(x_real, x_imag, fsl),
            "g_r": g_r_t, "g_i": g_i_t, "beta_row": beta_row,
            **consts,
        })

    res = run_bass_kernel_spmd(nc, in_maps, list(range(NCORES)))

    full = np.empty((B, C, F, 2), dtype=np.float32)
    for k in range(NCORES):
        full[:, :, FSH * k:FSH * (k + 1)] = (
            np.asarray(res.results[k]["out"]).astype(np.float32)
            .reshape(B, C, FSH, 2)
        )
    return full


# revision 31
# speedup vs baseline: 1.4463x; 1.0201x over previous
"""ComplexLayerNorm Trainium2 kernel (8 NeuronCores, SPMD, F-sharded).

Math (see reference): per-feature 2x2 covariance whitening of (re, im) over
all B*C samples (centered with the batch-only mean mu_b), after subtracting
the complex mean over F, plus complex affine.

v4 design:
  * F-sharding: each core owns 256 features (2 chunks of 128 on partitions)
    and ALL B*C = 8192 samples; the covariance path is fully core-local.
  * Only the per-sample complex mean over F crosses cores: partials are
    scaled x64, quantized to fp8e4 (plenty for a mean-subtraction term) and
    ride ONE AllGather that overlaps the apply phase.  The 8-way shard sum
    AND the beta add are folded into a K=17 correction matmul (16 gathered
    mean rows + a ones row), whose rhs carries -A/64 coefficients.
  * Engine assignment respects the cost model: ACT does Square+accum and
    the PSUM mean-staging copies; Pool does the cross products and the
    first T-tree fold (TensorTensor); DVE does the x_i^2 products (2x
    fp16 TT), all second-moment accumulations (4x fp16 TensorScalarPtr
    accum), and the deep T-tree folds.
  * Two-pass apply: pass 1 (collective-independent) does the diagonal-W
    x-matmuls into PSUM and stages uncorrected A@x; pass 2 adds the K=17
    correction (DVE stt from PSUM on even groups, ACT-copy + Pool add on
    odd groups) and stores.  Emission interleaves the passes so the PE
    never idles waiting for the collective.
"""

import numpy as np
import ml_dtypes

import bass_rust
import concourse.bass as bass
import concourse.mybir as mybir
from concourse import tile
from concourse.bass_utils import run_bass_kernel_spmd


def split_multi_waits(nc):
    """The walrus build in this container allows only ONE sync-wait command
    per instruction; Tile emits several.  Split extras into preceding
    single-wait NoOps on the same engine (sequential waits == AND)."""
    cnt = 0
    for bb in nc.main_func.blocks:
        il = bb.instructions
        newlist = []
        changed = False
        for inst in list(il):
            si = inst.sync_info
            waits = list(si.on_wait) if si else []
            if len(waits) > 1:
                changed = True
                for w in waits[:-1]:
                    cnt += 1
                    nop = bass_rust.InstNoOp(name=f"I-wsplit-{cnt}")
                    nop.engine = inst.engine
                    nop.sync_info = mybir.SyncInfo(on_wait=[w], on_update=[])
                    newlist.append(nop)
                inst.sync_info = mybir.SyncInfo(
                    on_wait=[waits[-1]], on_update=list(si.on_update))
            newlist.append(inst)
        if changed:
            il[:] = newlist
    return cnt

FP = mybir.dt.float32
FR = mybir.dt.float32r
F16 = mybir.dt.float16
F8 = mybir.dt.float8e4
AF = mybir.ActivationFunctionType
OP = mybir.AluOpType
AX = mybir.AxisListType

B, C, F = 64, 128, 2048
NCORES = 8
FSH = F // NCORES           # 256 features per core
NCH = FSH // 128            # 2 f-chunks of 128 (on partitions)
BC = B * C                  # 8192 samples per core (full batch)
NBB = 4                     # bigblocks of 2048 samples for DMA streaming
BBS = BC // NBB             # 2048
NG = BC // 128              # 64 apply groups (128 samples each)
EPS = 1e-4
NM1 = float(B * C - 1)      # 8191
MSCALE = 64.0               # fp8 mean-partial scaling (folded into consts)


def build_bass():
    nc = bass.Bass()

    # x, fp16, f-on-partitions: xt[p, 16384*comp + 8192*cc + j]
    #   = x_comp[sample j, f_local = 128*cc + p]
    xt = nc.dram_tensor("xt", [128, 2 * NCH * BC], F16, kind="ExternalInput")
    ident = nc.dram_tensor("ident", [128, 128], FP, kind="ExternalInput")
    # mean-matmul weights: MSCALE/F
    onesF = nc.dram_tensor("onesF", [128, 1], F16, kind="ExternalInput")
    # even/odd row selectors for the Arow build
    onesEv = nc.dram_tensor("onesEv", [128, 16], F16, kind="ExternalInput")
    onesOd = nc.dram_tensor("onesOd", [128, 16], F16, kind="ExternalInput")
    # gamma for this core's shard, f-on-partitions: [128, NCH]
    g_r = nc.dram_tensor("g_r", [128, NCH], FP, kind="ExternalInput")
    g_i = nc.dram_tensor("g_i", [128, NCH], FP, kind="ExternalInput")
    # beta for this shard (fp8), apply-column order: [0, 256*cc + 2*g + c]
    beta_row = nc.dram_tensor("beta_row", [1, 2 * FSH], F8,
                              kind="ExternalInput")
    onesrow = nc.dram_tensor("onesrow", [1, BC], F8, kind="ExternalInput")

    out = nc.dram_tensor("out", [BC, 2 * FSH], F16, kind="ExternalOutput")

    with tile.TileContext(nc) as tc:
        with (
            tc.tile_pool(name="big", bufs=1) as big,
            tc.tile_pool(name="small", bufs=1) as small,
            tc.tile_pool(name="stg", bufs=1) as stgp,
            tc.tile_pool(name="dram", bufs=1, space="DRAM") as dram,
        ):
            # ---- constants (DMAs issued after the first x block; onesF
            # rides the scalar queue since the mean matmuls need it early)
            ident_t = small.tile([128, 128], FP, tag="ident")
            onesF_t = small.tile([128, 1], F16, tag="onesF")
            onesEv_t = small.tile([128, 16], F16, tag="onesEv")
            onesOd_t = small.tile([128, 16], F16, tag="onesOd")
            g_r_t = small.tile([128, NCH], FP, tag="g_r")
            g_i_t = small.tile([128, NCH], FP, tag="g_i")

            def emit_const_dmas():
                nc.scalar.dma_start(onesF_t[:], onesF[:])
                nc.scalar.dma_start(ident_t[:], ident[:])
                nc.scalar.dma_start(onesEv_t[:], onesEv[:])
                nc.scalar.dma_start(onesOd_t[:], onesOd[:])
                nc.scalar.dma_start(g_r_t[:], g_r[:])
                nc.scalar.dma_start(g_i_t[:], g_i[:])

            # mg: 16 gathered fp8 mean-partial rows + ones row (for beta)
            mg = small.tile([17, BC], F8, tag="mg")
            # Arow17: correction-matmul rhs (fp8); row 16 = beta
            Arow = small.tile([17, 2 * FSH], F8, tag="Arow")

            # ---- persistent x (fp16, f-on-partitions)
            xT = big.tile([128, 2 * NCH * BC], F16, tag="xT")

            # batch-sums over b: T_sb[p, 128*(2*cc+comp) + c]
            T_sb = small.tile([128, 2 * NCH * C], F16, tag="T_sb")
            # second-moment accumulator columns: 8 cols (cc,b) per
            # moment m (0=rr, 1=ii, 2=ri)
            S_acc = small.tile([128, 24], FP, tag="S_acc")

            from contextlib import ExitStack
            _stk = ExitStack()
            scr = _stk.enter_context(tc.tile_pool(name="scr", bufs=1))
            sc2 = _stk.enter_context(tc.tile_pool(name="sc2", bufs=2))
            trp = _stk.enter_context(tc.tile_pool(name="trp", bufs=2))
            msp = _stk.enter_context(tc.tile_pool(name="msp", bufs=2))
            ps_mean = _stk.enter_context(
                tc.tile_pool(name="ps_mean", bufs=2, space="PSUM"))
            ar_in = dram.tile([2, BC], F8, tag="ar_in")
            ar_out = dram.tile([16, BC], F8, tag="ar_out")

            def xsl(comp, cc, lo, n):
                return xT[:, 16384 * comp + BC * cc + lo:
                          16384 * comp + BC * cc + lo + n]

            def xdr(comp, cc, lo, n):
                return xt[:, 16384 * comp + BC * cc + lo:
                          16384 * comp + BC * cc + lo + n]

            # ---- Phase A: stream bigblocks; stats overlap the DMA
            prod_i = {}
            for b in range(NBB):
                lo = BBS * b
                h2, hb = b // 2, b % 2
                for cc in range(NCH):
                    for comp in range(2):
                        nc.sync.dma_start(xsl(comp, cc, lo, BBS),
                                          xdr(comp, cc, lo, BBS))
                    if b == 0 and cc == 0:
                        emit_const_dmas()
                    xr = xsl(0, cc, lo, BBS)
                    xi = xsl(1, cc, lo, BBS)
                    # S_rr on ACT: Square w/ accumulate (2048 grain)
                    sa = scr.tile([128, BBS], F16, tag="sq_act")
                    nc.scalar.activation(
                        sa[:], xr, AF.Square,
                        accum_out=S_acc[:, cc * 4 + b:cc * 4 + b + 1])
                    # x_i^2 products on DVE (2x fp16 TT) + 4x TSP accum
                    pi = scr.tile([128, BBS], F16, tag="prod_i",
                                  name=f"prod_i_{cc}_{b}")
                    nc.vector.tensor_tensor(out=pi[:], in0=xi, in1=xi,
                                            op=OP.mult)
                    nc.vector.tensor_scalar(
                        out=pi[:], in0=pi[:], scalar1=1.0, scalar2=0.0,
                        op0=OP.mult, op1=OP.add,
                        accum_out=S_acc[:, 8 + cc * 4 + b:
                                        8 + cc * 4 + b + 1])
                    # cross products on Pool, accum on DVE (4x TSP)
                    px = sc2.tile([128, BBS], F16, tag="prod_x")
                    nc.gpsimd.tensor_tensor(out=px[:], in0=xr, in1=xi,
                                            op=OP.mult)
                    nc.vector.tensor_scalar(
                        out=px[:], in0=px[:], scalar1=1.0, scalar2=0.0,
                        op0=OP.mult, op1=OP.add,
                        accum_out=S_acc[:, 16 + cc * 4 + b:
                                        16 + cc * 4 + b + 1])
                    # mean-partial matmuls: 512-block m of this bigblock
                    # lands on psum partition 64*(m%2), col 512*(m//2).
                    for comp in range(2):
                        if cc == 0 and comp == 0:
                            pm = [ps_mean.tile([128, 1024], FP,
                                               tag=f"psm{c2}",
                                               name=f"psm{c2}_{b}")
                                  for c2 in range(2)]
                            prod_i[("pm", b)] = pm
                        pm = prod_i[("pm", b)]
                        for m in range(4):
                            nc.tensor.matmul(
                                pm[comp][64 * (m % 2):64 * (m % 2) + 1,
                                         512 * (m // 2):512 * (m // 2) + 512],
                                onesF_t[:],
                                xsl(comp, cc, lo + 512 * m, 512),
                                start=(cc == 0), stop=(cc == NCH - 1),
                            )
                # T-tree at 2048 grain: fold1 on Pool, rest on DVE
                for cc in range(NCH):
                    for comp in range(2):
                        t1 = trp.tile([128, BBS // 2], F16, tag="t1")
                        nc.gpsimd.tensor_tensor(
                            out=t1[:], in0=xsl(comp, cc, lo, BBS // 2),
                            in1=xsl(comp, cc, lo + BBS // 2, BBS // 2),
                            op=OP.add)
                        t2 = trp.tile([128, BBS // 4], F16, tag="t2")
                        nc.vector.tensor_tensor(
                            out=t2[:], in0=t1[:, 0:BBS // 4],
                            in1=t1[:, BBS // 4:BBS // 2], op=OP.add)
                        t3 = trp.tile([128, BBS // 8], F16, tag="t3")
                        nc.vector.tensor_tensor(
                            out=t3[:], in0=t2[:, 0:BBS // 8],
                            in1=t2[:, BBS // 8:BBS // 4], op=OP.add)
                        td = T_sb[:, C * (2 * cc + comp):
                                  C * (2 * cc + comp) + C]
                        if b == 0:
                            nc.vector.tensor_tensor(
                                out=td, in0=t3[:, 0:C], in1=t3[:, C:2 * C],
                                op=OP.add)
                        else:
                            t4 = trp.tile([128, C], F16, tag="t4")
                            nc.vector.tensor_tensor(
                                out=t4[:], in0=t3[:, 0:C], in1=t3[:, C:2 * C],
                                op=OP.add)
                            nc.vector.tensor_tensor(
                                out=td, in0=td, in1=t4[:], op=OP.add)
                # stage mean partials PSUM -> SBUF (fp8) and drain to DRAM:
                # ar_in[comp, 2048*b + 1024*h + 512*m' + j] where data rows
                # sit at psum partitions {0,64} (m'), cols 512*h.
                pm = prod_i[("pm", b)]
                for comp in range(2):
                    mt = msp.tile([128, 1024], F8, tag=f"ms{comp}",
                                  name=f"ms{comp}_{b}")
                    nc.scalar.copy(mt[:], pm[comp][:])
                    src = mt[:].rearrange(
                        "(m o) (h j) -> m o h j", o=64, h=2)[:, 0:1]
                    dst = ar_in[comp:comp + 1,
                                BBS * b:BBS * (b + 1)].rearrange(
                        "o (h m j) -> m o h j", h=2, m=2)
                    nc.sync.dma_start(dst, src)

            # ---- AllGather the fp8 mean partials (overlaps everything below)
            nc.gpsimd.collective_compute(
                "AllGather", OP.bypass,
                replica_groups=[list(range(NCORES))],
                ins=[ar_in.opt()],
                outs=[ar_out.opt()],
            )
            # constants for pass 2 (DMA during the collective window)
            nc.scalar.dma_start(mg[16:17, :], onesrow[:])
            nc.scalar.dma_start(Arow[16:17, :], beta_row[:])
            # gather result in two halves so early groups start sooner
            for hf in range(2):
                nc.sync.dma_start(
                    mg[0:16, 4096 * hf:4096 * (hf + 1)],
                    ar_out[:, 4096 * hf:4096 * (hf + 1)])

            # ---- stats finalize: S6 (cols m*2+cc), corr6, cov6
            S6 = small.tile([128, 6], FP, tag="S6")
            nc.vector.tensor_reduce(
                S6[:, 0:2], S_acc[:, 0:8].rearrange("p (g b) -> p g b", b=4),
                AX.X, OP.add)
            nc.vector.tensor_reduce(
                S6[:, 2:6], S_acc[:, 8:24].rearrange("p (g b) -> p g b", b=4),
                AX.X, OP.add)
            corr6 = small.tile([128, 6], FP, tag="corr6")
            for m, (ca, cb) in enumerate(((0, 0), (1, 1), (0, 1))):
                for cc in range(NCH):
                    pr = sc2.tile([128, C], F16, tag="prod_x",
                                  name=f"tt_{m}_{cc}")
                    nc.vector.scalar_tensor_tensor(
                        out=pr[:],
                        in0=T_sb[:, C * (2 * cc + ca):C * (2 * cc + ca) + C],
                        scalar=1.0,
                        in1=T_sb[:, C * (2 * cc + cb):C * (2 * cc + cb) + C],
                        op0=OP.mult, op1=OP.mult,
                        accum_out=corr6[:, m * 2 + cc:m * 2 + cc + 1])
            cov6 = small.tile([128, 6], FP, tag="cov6")
            nc.vector.scalar_tensor_tensor(
                out=cov6[:], in0=corr6[:], scalar=-1.0 / B, in1=S6[:],
                op0=OP.mult, op1=OP.add)
            nc.vector.tensor_scalar(
                out=cov6[:], in0=cov6[:], scalar1=1.0 / NM1, scalar2=None,
                op0=OP.mult)
            _stk.close()  # release scratch + mean PSUM

            # ---- Phase C: closed-form 2x2 inverse sqrt, fold gamma
            def stile(tag):
                return small.tile([128, NCH], FP, tag=tag, name=tag)

            arr, cii, bri = stile("arr"), stile("cii"), stile("bri")
            nc.vector.tensor_scalar(out=arr[:], in0=cov6[:, 0:2],
                                    scalar1=EPS, scalar2=None, op0=OP.add)
            nc.vector.tensor_scalar(out=cii[:], in0=cov6[:, 2:4],
                                    scalar1=EPS, scalar2=None, op0=OP.add)
            nc.vector.tensor_copy(bri[:], cov6[:, 4:6])

            det, tmp = stile("det"), stile("tmp")
            nc.vector.tensor_tensor(out=det[:], in0=arr[:], in1=cii[:],
                                    op=OP.mult)
            nc.vector.tensor_tensor(out=tmp[:], in0=bri[:], in1=bri[:],
                                    op=OP.mult)
            nc.vector.tensor_tensor(out=det[:], in0=det[:], in1=tmp[:],
                                    op=OP.subtract)
            s_t = stile("s_t")
            nc.scalar.activation(s_t[:], det[:], AF.Sqrt)
            tsum = stile("tsum")
            nc.vector.tensor_tensor(out=tsum[:], in0=arr[:], in1=cii[:],
                                    op=OP.add)
            nc.vector.scalar_tensor_tensor(out=tsum[:], in0=s_t[:], scalar=2.0,
                                           in1=tsum[:], op0=OP.mult,
                                           op1=OP.add)
            tval = stile("tval")
            nc.scalar.activation(tval[:], tsum[:], AF.Sqrt)
            den, rden = stile("den"), stile("rden")
            nc.vector.tensor_tensor(out=den[:], in0=s_t[:], in1=tval[:],
                                    op=OP.mult)
            nc.vector.reciprocal(rden[:], den[:])

            w_rr, w_ii, wri = stile("w_rr"), stile("w_ii"), stile("wri")
            nc.vector.tensor_tensor(out=w_rr[:], in0=cii[:], in1=s_t[:],
                                    op=OP.add)
            nc.vector.tensor_tensor(out=w_rr[:], in0=w_rr[:], in1=rden[:],
                                    op=OP.mult)
            nc.vector.tensor_tensor(out=w_ii[:], in0=arr[:], in1=s_t[:],
                                    op=OP.add)
            nc.vector.tensor_tensor(out=w_ii[:], in0=w_ii[:], in1=rden[:],
                                    op=OP.mult)
            nc.vector.tensor_tensor(out=wri[:], in0=bri[:], in1=rden[:],
                                    op=OP.mult)
            nc.vector.tensor_scalar(out=wri[:], in0=wri[:], scalar1=-1.0,
                                    scalar2=None, op0=OP.mult)

            # A = G @ W
            a_rr, a_ri = stile("a_rr"), stile("a_ri")
            a_ir, a_ii = stile("a_ir"), stile("a_ii")
            u, v = stile("u"), stile("v")
            for dst, (wa, wb) in ((a_rr, (w_rr, wri)), (a_ri, (wri, w_ii))):
                nc.vector.tensor_tensor(out=u[:], in0=g_r_t[:], in1=wa[:],
                                        op=OP.mult)
                nc.vector.tensor_tensor(out=v[:], in0=g_i_t[:], in1=wb[:],
                                        op=OP.mult)
                nc.vector.tensor_tensor(out=dst[:], in0=u[:], in1=v[:],
                                        op=OP.subtract)
            for dst, (wa, wb) in ((a_ir, (w_rr, wri)), (a_ii, (wri, w_ii))):
                nc.vector.tensor_tensor(out=u[:], in0=g_i_t[:], in1=wa[:],
                                        op=OP.mult)
                nc.vector.tensor_tensor(out=v[:], in0=g_r_t[:], in1=wb[:],
                                        op=OP.mult)
                nc.vector.tensor_tensor(out=dst[:], in0=u[:], in1=v[:],
                                        op=OP.add)

            # ---- W tiles (fp16), diagonal per chunk: W[p, 2g+c]
            Ws = []
            for cc in range(NCH):
                W_r = small.tile([128, 256], F16, tag=f"W_r{cc}",
                                 name=f"W_r{cc}")
                W_i = small.tile([128, 256], F16, tag=f"W_i{cc}",
                                 name=f"W_i{cc}")
                for W, (ev, od) in ((W_r, (a_rr, a_ir)), (W_i, (a_ri, a_ii))):
                    Wv = W[:].rearrange("p (g c) -> p g c", c=2)
                    nc.vector.tensor_scalar(
                        out=Wv[:, :, 0], in0=ident_t[:],
                        scalar1=ev[:, cc:cc + 1], scalar2=None, op0=OP.mult)
                    nc.vector.tensor_scalar(
                        out=Wv[:, :, 1], in0=ident_t[:],
                        scalar1=od[:, cc:cc + 1], scalar2=None, op0=OP.mult)
                Ws.append((W_r, W_i))

            # ---- Arow rows 0..15: -A/MSCALE coefs in apply-column order,
            # via matmuls against the W tiles with even/odd selectors.
            from contextlib import ExitStack as _ES2
            _stk2 = _ES2()
            ps_t = _stk2.enter_context(
                tc.tile_pool(name="ps_t", bufs=1, space="PSUM"))
            psA = ps_t.tile([16, 2 * FSH], FP, tag="psA")
            for cc in range(NCH):
                W_r, W_i = Ws[cc]
                nc.tensor.matmul(psA[:, 256 * cc:256 * (cc + 1)],
                                 onesEv_t[:], W_r[:], start=True, stop=False)
                nc.tensor.matmul(psA[:, 256 * cc:256 * (cc + 1)],
                                 onesOd_t[:], W_i[:], start=False, stop=True)
            nc.vector.tensor_scalar(out=Arow[0:16, :], in0=psA[:],
                                    scalar1=-1.0 / MSCALE, scalar2=None,
                                    op0=OP.mult)

            # ---- Phase D: two-pass apply with interleaved emission
            stg = stgp.tile([128, NG * 512], F16, tag="stg")
            _stk3 = _ES2()
            ps_o = _stk3.enter_context(
                tc.tile_pool(name="ps_o", bufs=6, space="PSUM"))
            corrp = _stk3.enter_context(tc.tile_pool(name="corrp", bufs=2))

            def pass1(g):
                po = ps_o.tile([128, 512], FP, tag="po", name=f"po1_{g}")
                for cc in range(NCH):
                    W_r, W_i = Ws[cc]
                    nc.tensor.matmul(
                        po[:, 256 * cc:256 * (cc + 1)],
                        xsl(0, cc, 128 * g, 128), W_r[:],
                        start=True, stop=False)
                    nc.tensor.matmul(
                        po[:, 256 * cc:256 * (cc + 1)],
                        xsl(1, cc, 128 * g, 128), W_i[:],
                        start=False, stop=True)
                dst = stg[:, 512 * g:512 * (g + 1)]
                if g % 2 == 0:
                    nc.vector.tensor_copy(dst, po[:])
                else:
                    nc.scalar.copy(dst, po[:])

            def pass2(g):
                po = ps_o.tile([128, 512], FP, tag="po", name=f"po2_{g}")
                nc.tensor.matmul(
                    po[:], mg[:, 128 * g:128 * (g + 1)], Arow[:],
                    start=True, stop=True)
                dst = stg[:, 512 * g:512 * (g + 1)]
                if g % 2 == 0:
                    nc.vector.scalar_tensor_tensor(
                        out=dst, in0=dst, scalar=1.0, in1=po[:],
                        op0=OP.mult, op1=OP.add)
                else:
                    ct = corrp.tile([128, 512], F16, tag="ct")
                    nc.scalar.copy(ct[:], po[:])
                    nc.gpsimd.tensor_tensor(
                        out=dst, in0=dst, in1=ct[:], op=OP.add)
                if g % 4 == 3:
                    g0 = g - 3
                    dstd = out.rearrange("(a p) f -> p a f", p=128)[
                        :, g0:g0 + 4, :]
                    src = stg[:, 512 * g0:512 * (g + 1)].rearrange(
                        "p (a q) -> p a q", q=512)
                    if (g // 4) % 2 == 0:
                        nc.sync.dma_start(dstd, src)
                    else:
                        nc.scalar.dma_start(dstd, src)

            for g in range(NG):
                pass1(g)
            for g in range(NG):
                pass2(g)
            _stk3.close()
            _stk2.close()

    split_multi_waits(nc)
    return nc


_CACHE = {}


def _get_nc():
    if "nc" not in _CACHE:
        _CACHE["nc"] = build_bass()
    return _CACHE["nc"]


def _constants():
    if "consts" not in _CACHE:
        _CACHE["consts"] = {
            "ident": np.eye(128, dtype=np.float32),
            "onesF": np.full((128, 1), MSCALE / F, dtype=np.float16),
            "onesrow": np.ones((1, BC), dtype=ml_dtypes.float8_e4m3),
            "onesEv": np.tile((np.arange(16) % 2 == 0).astype(np.float16),
                              (128, 1)),
            "onesOd": np.tile((np.arange(16) % 2 == 1).astype(np.float16),
                              (128, 1)),
        }
    return _CACHE["consts"]


def _host_xt(xr, xi, fsl):
    """Build xt[p, 16384*comp + 8192*cc + j] = x_comp[j, 128*cc + p]."""
    halves = []
    for x in (xr, xi):
        xs = x[:, fsl].reshape(BC, NCH, 128)        # (j, cc, p)
        halves.append(np.transpose(xs, (2, 1, 0)).reshape(128, NCH * BC))
    return np.ascontiguousarray(
        np.concatenate(halves, axis=1)).astype(np.float16)


def kernel(x_real, x_imag, gamma_r, gamma_i, beta_r, beta_i):
    x_real = np.asarray(x_real, dtype=np.float32).reshape(BC, F)
    x_imag = np.asarray(x_imag, dtype=np.float32).reshape(BC, F)
    gamma_r = np.asarray(gamma_r, dtype=np.float32)
    gamma_i = np.asarray(gamma_i, dtype=np.float32)
    beta_r = np.asarray(beta_r, dtype=np.float32)
    beta_i = np.asarray(beta_i, dtype=np.float32)

    nc = _get_nc()
    consts = _constants()

    in_maps = []
    for k in range(NCORES):
        fsl = slice(FSH * k, FSH * (k + 1))
        g_r_t = np.ascontiguousarray(gamma_r[fsl].reshape(NCH, 128).T)
        g_i_t = np.ascontiguousarray(gamma_i[fsl].reshape(NCH, 128).T)
        beta_row = np.ascontiguousarray(
            np.stack([beta_r[fsl], beta_i[fsl]], axis=-1).reshape(1, 2 * FSH)
        ).astype(ml_dtypes.float8_e4m3)
        in_maps.append({
            "xt": _host_xt(x_real, x_imag, fsl),
            "g_r": g_r_t, "g_i": g_i_t, "beta_row": beta_row,
            **consts,
        })

    res = run_bass_kernel_spmd(nc, in_maps, list(range(NCORES)))

    full = np.empty((B, C, F, 2), dtype=np.float32)
    for k in range(NCORES):
        full[:, :, FSH * k:FSH * (k + 1)] = (
            np.asarray(res.results[k]["out"]).astype(np.float32)
            .reshape(B, C, FSH, 2)
        )
    return full


# revision 33
# speedup vs baseline: 1.4961x; 1.0345x over previous
"""ComplexLayerNorm Trainium2 kernel (8 NeuronCores, SPMD, F-sharded).

Math (see reference): per-feature 2x2 covariance whitening of (re, im) over
all B*C samples (centered with the batch-only mean mu_b), after subtracting
the complex mean over F, plus complex affine.

v4 design:
  * F-sharding: each core owns 256 features (2 chunks of 128 on partitions)
    and ALL B*C = 8192 samples; the covariance path is fully core-local.
  * Only the per-sample complex mean over F crosses cores: partials are
    scaled x64, quantized to fp8e4 (plenty for a mean-subtraction term) and
    ride ONE AllGather that overlaps the apply phase.  The 8-way shard sum
    AND the beta add are folded into a K=17 correction matmul (16 gathered
    mean rows + a ones row), whose rhs carries -A/64 coefficients.
  * Engine assignment respects the cost model: ACT does Square+accum and
    the PSUM mean-staging copies; Pool does the cross products and the
    first T-tree fold (TensorTensor); DVE does the x_i^2 products (2x
    fp16 TT), all second-moment accumulations (4x fp16 TensorScalarPtr
    accum), and the deep T-tree folds.
  * Two-pass apply: pass 1 (collective-independent) does the diagonal-W
    x-matmuls into PSUM and stages uncorrected A@x; pass 2 adds the K=17
    correction (DVE stt from PSUM on even groups, ACT-copy + Pool add on
    odd groups) and stores.  Emission interleaves the passes so the PE
    never idles waiting for the collective.
"""

import numpy as np
import ml_dtypes

import bass_rust
import concourse.bass as bass
import concourse.mybir as mybir
from concourse import tile
from concourse.bass_utils import run_bass_kernel_spmd


def split_multi_waits(nc):
    """The walrus build in this container allows only ONE sync-wait command
    per instruction; Tile emits several.  Split extras into preceding
    single-wait NoOps on the same engine (sequential waits == AND)."""
    cnt = 0
    for bb in nc.main_func.blocks:
        il = bb.instructions
        newlist = []
        changed = False
        for inst in list(il):
            si = inst.sync_info
            waits = list(si.on_wait) if si else []
            if len(waits) > 1:
                changed = True
                for w in waits[:-1]:
                    cnt += 1
                    nop = bass_rust.InstNoOp(name=f"I-wsplit-{cnt}")
                    nop.engine = inst.engine
                    nop.sync_info = mybir.SyncInfo(on_wait=[w], on_update=[])
                    newlist.append(nop)
                inst.sync_info = mybir.SyncInfo(
                    on_wait=[waits[-1]], on_update=list(si.on_update))
            newlist.append(inst)
        if changed:
            il[:] = newlist
    return cnt

FP = mybir.dt.float32
FR = mybir.dt.float32r
F16 = mybir.dt.float16
F8 = mybir.dt.float8e4
AF = mybir.ActivationFunctionType
OP = mybir.AluOpType
AX = mybir.AxisListType

B, C, F = 64, 128, 2048
NCORES = 8
FSH = F // NCORES           # 256 features per core
NCH = FSH // 128            # 2 f-chunks of 128 (on partitions)
BC = B * C                  # 8192 samples per core (full batch)
NBB = 4                     # bigblocks of 2048 samples for DMA streaming
BBS = BC // NBB             # 2048
NG = BC // 128              # 64 apply groups (128 samples each)
EPS = 1e-4
NM1 = float(B * C - 1)      # 8191
MSCALE = 64.0               # fp8 mean-partial scaling (folded into consts)


def build_bass():
    nc = bass.Bass()

    # x, fp16, f-on-partitions: xt[p, 16384*comp + 8192*cc + j]
    #   = x_comp[sample j, f_local = 128*cc + p]
    xt = nc.dram_tensor("xt", [128, 2 * NCH * BC], F16, kind="ExternalInput")
    ident = nc.dram_tensor("ident", [128, 128], FP, kind="ExternalInput")
    # mean-matmul weights: MSCALE/F
    onesF = nc.dram_tensor("onesF", [128, 1], F16, kind="ExternalInput")
    # even/odd row selectors for the Arow build
    onesEv = nc.dram_tensor("onesEv", [128, 16], F16, kind="ExternalInput")
    onesOd = nc.dram_tensor("onesOd", [128, 16], F16, kind="ExternalInput")
    # gamma for this core's shard, f-on-partitions: [128, NCH]
    g_r = nc.dram_tensor("g_r", [128, NCH], FP, kind="ExternalInput")
    g_i = nc.dram_tensor("g_i", [128, NCH], FP, kind="ExternalInput")
    # beta for this shard (fp8), apply-column order: [0, 256*cc + 2*g + c]
    beta_row = nc.dram_tensor("beta_row", [1, 2 * FSH], F8,
                              kind="ExternalInput")
    onesrow = nc.dram_tensor("onesrow", [1, BC], F8, kind="ExternalInput")

    out = nc.dram_tensor("out", [BC, 2 * FSH], F16, kind="ExternalOutput")

    with tile.TileContext(nc) as tc:
        with (
            tc.tile_pool(name="big", bufs=1) as big,
            tc.tile_pool(name="small", bufs=1) as small,
            tc.tile_pool(name="stg", bufs=1) as stgp,
            tc.tile_pool(name="dram", bufs=1, space="DRAM") as dram,
        ):
            # ---- constants (DMAs issued after the first x block; onesF
            # rides the scalar queue since the mean matmuls need it early)
            ident_t = small.tile([128, 128], FP, tag="ident")
            onesF_t = small.tile([128, 1], F16, tag="onesF")
            onesEv_t = small.tile([128, 16], F16, tag="onesEv")
            onesOd_t = small.tile([128, 16], F16, tag="onesOd")
            g_r_t = small.tile([128, NCH], FP, tag="g_r")
            g_i_t = small.tile([128, NCH], FP, tag="g_i")

            def emit_const_dmas():
                nc.scalar.dma_start(onesF_t[:], onesF[:])
                nc.scalar.dma_start(ident_t[:], ident[:])
                nc.scalar.dma_start(onesEv_t[:], onesEv[:])
                nc.scalar.dma_start(onesOd_t[:], onesOd[:])
                nc.scalar.dma_start(g_r_t[:], g_r[:])
                nc.scalar.dma_start(g_i_t[:], g_i[:])

            # mg: 16 gathered fp8 mean-partial rows + ones row (for beta)
            mg = small.tile([17, BC], F8, tag="mg")
            # Arow17: correction-matmul rhs (fp8); row 16 = beta
            Arow = small.tile([17, 2 * FSH], F8, tag="Arow")

            # ---- persistent x (fp16, f-on-partitions)
            xT = big.tile([128, 2 * NCH * BC], F16, tag="xT")

            # batch-sums over b: T_sb[p, 128*(2*cc+comp) + c]
            T_sb = small.tile([128, 2 * NCH * C], F16, tag="T_sb")
            # second-moment accumulator columns: 8 cols (cc,b) per
            # moment m (0=rr, 1=ii, 2=ri)
            S_acc = small.tile([128, 24], FP, tag="S_acc")

            from contextlib import ExitStack
            _stk = ExitStack()
            scr = _stk.enter_context(tc.tile_pool(name="scr", bufs=1))
            sc2 = _stk.enter_context(tc.tile_pool(name="sc2", bufs=2))
            trp = _stk.enter_context(tc.tile_pool(name="trp", bufs=2))
            msp = _stk.enter_context(tc.tile_pool(name="msp", bufs=2))
            ps_mean = _stk.enter_context(
                tc.tile_pool(name="ps_mean", bufs=2, space="PSUM"))
            ar_in = dram.tile([2, BC], F8, tag="ar_in")
            ar_out = dram.tile([16, BC], F8, tag="ar_out")

            def xsl(comp, cc, lo, n):
                return xT[:, 16384 * comp + BC * cc + lo:
                          16384 * comp + BC * cc + lo + n]

            def xdr(comp, cc, lo, n):
                return xt[:, 16384 * comp + BC * cc + lo:
                          16384 * comp + BC * cc + lo + n]

            # ---- Phase A: stream bigblocks; stats overlap the DMA
            prod_i = {}
            for b in range(NBB):
                lo = BBS * b
                h2, hb = b // 2, b % 2
                for cc in range(NCH):
                    for comp in range(2):
                        nc.sync.dma_start(xsl(comp, cc, lo, BBS),
                                          xdr(comp, cc, lo, BBS))
                    if b == 0 and cc == 0:
                        emit_const_dmas()
                    xr = xsl(0, cc, lo, BBS)
                    xi = xsl(1, cc, lo, BBS)
                    # S_rr on ACT: Square w/ accumulate (2048 grain)
                    sa = scr.tile([128, BBS], F16, tag="sq_act")
                    nc.scalar.activation(
                        sa[:], xr, AF.Square,
                        accum_out=S_acc[:, cc * 4 + b:cc * 4 + b + 1])
                    # x_i^2 products on DVE (2x fp16 TT) + 4x TSP accum
                    pi = scr.tile([128, BBS], F16, tag="prod_i",
                                  name=f"prod_i_{cc}_{b}")
                    nc.vector.tensor_tensor(out=pi[:], in0=xi, in1=xi,
                                            op=OP.mult)
                    nc.vector.tensor_scalar(
                        out=pi[:], in0=pi[:], scalar1=1.0, scalar2=0.0,
                        op0=OP.mult, op1=OP.add,
                        accum_out=S_acc[:, 8 + cc * 4 + b:
                                        8 + cc * 4 + b + 1])
                    # cross products on Pool, accum on DVE (4x TSP)
                    px = sc2.tile([128, BBS], F16, tag="prod_x")
                    nc.gpsimd.tensor_tensor(out=px[:], in0=xr, in1=xi,
                                            op=OP.mult)
                    nc.vector.tensor_scalar(
                        out=px[:], in0=px[:], scalar1=1.0, scalar2=0.0,
                        op0=OP.mult, op1=OP.add,
                        accum_out=S_acc[:, 16 + cc * 4 + b:
                                        16 + cc * 4 + b + 1])
                    # mean-partial matmuls: 512-block m of this bigblock
                    # lands on psum partition 64*(m%2), col 512*(m//2).
                    for comp in range(2):
                        if cc == 0 and comp == 0:
                            pm = [ps_mean.tile([128, 1024], FP,
                                               tag=f"psm{c2}",
                                               name=f"psm{c2}_{b}")
                                  for c2 in range(2)]
                            prod_i[("pm", b)] = pm
                        pm = prod_i[("pm", b)]
                        for m in range(4):
                            nc.tensor.matmul(
                                pm[comp][64 * (m % 2):64 * (m % 2) + 1,
                                         512 * (m // 2):512 * (m // 2) + 512],
                                onesF_t[:],
                                xsl(comp, cc, lo + 512 * m, 512),
                                start=(cc == 0), stop=(cc == NCH - 1),
                            )
                # T-tree at 2048 grain: fold1 on Pool, rest on DVE
                for cc in range(NCH):
                    for comp in range(2):
                        t1 = trp.tile([128, BBS // 2], F16, tag="t1")
                        nc.gpsimd.tensor_tensor(
                            out=t1[:], in0=xsl(comp, cc, lo, BBS // 2),
                            in1=xsl(comp, cc, lo + BBS // 2, BBS // 2),
                            op=OP.add)
                        t2 = trp.tile([128, BBS // 4], F16, tag="t2")
                        nc.vector.tensor_tensor(
                            out=t2[:], in0=t1[:, 0:BBS // 4],
                            in1=t1[:, BBS // 4:BBS // 2], op=OP.add)
                        t3 = trp.tile([128, BBS // 8], F16, tag="t3")
                        nc.vector.tensor_tensor(
                            out=t3[:], in0=t2[:, 0:BBS // 8],
                            in1=t2[:, BBS // 8:BBS // 4], op=OP.add)
                        td = T_sb[:, C * (2 * cc + comp):
                                  C * (2 * cc + comp) + C]
                        if b == 0:
                            nc.vector.tensor_tensor(
                                out=td, in0=t3[:, 0:C], in1=t3[:, C:2 * C],
                                op=OP.add)
                        else:
                            t4 = trp.tile([128, C], F16, tag="t4")
                            nc.vector.tensor_tensor(
                                out=t4[:], in0=t3[:, 0:C], in1=t3[:, C:2 * C],
                                op=OP.add)
                            nc.vector.tensor_tensor(
                                out=td, in0=td, in1=t4[:], op=OP.add)
                # stage mean partials PSUM -> SBUF (fp8) and drain to DRAM:
                # ar_in[comp, 2048*b + 1024*h + 512*m' + j] where data rows
                # sit at psum partitions {0,64} (m'), cols 512*h.
                pm = prod_i[("pm", b)]
                for comp in range(2):
                    mt = msp.tile([128, 1024], F8, tag=f"ms{comp}",
                                  name=f"ms{comp}_{b}")
                    nc.scalar.copy(mt[:], pm[comp][:])
                    src = mt[:].rearrange(
                        "(m o) (h j) -> m o h j", o=64, h=2)[:, 0:1]
                    dst = ar_in[comp:comp + 1,
                                BBS * b:BBS * (b + 1)].rearrange(
                        "o (h m j) -> m o h j", h=2, m=2)
                    nc.sync.dma_start(dst, src)

            # ---- AllGather the fp8 mean partials (overlaps everything below)
            nc.gpsimd.collective_compute(
                "AllGather", OP.bypass,
                replica_groups=[list(range(NCORES))],
                ins=[ar_in.opt()],
                outs=[ar_out.opt()],
            )
            # constants for pass 2 (DMA during the collective window)
            nc.scalar.dma_start(mg[16:17, :], onesrow[:])
            nc.scalar.dma_start(Arow[16:17, :], beta_row[:])
            # gather result in two halves so early groups start sooner
            for hf in range(2):
                nc.sync.dma_start(
                    mg[0:16, 4096 * hf:4096 * (hf + 1)],
                    ar_out[:, 4096 * hf:4096 * (hf + 1)])

            # ---- stats finalize: S6 (cols m*2+cc), corr6, cov6
            S6 = small.tile([128, 6], FP, tag="S6")
            nc.vector.tensor_reduce(
                S6[:, 0:2], S_acc[:, 0:8].rearrange("p (g b) -> p g b", b=4),
                AX.X, OP.add)
            nc.vector.tensor_reduce(
                S6[:, 2:6], S_acc[:, 8:24].rearrange("p (g b) -> p g b", b=4),
                AX.X, OP.add)
            corr6 = small.tile([128, 6], FP, tag="corr6")
            for m, (ca, cb) in enumerate(((0, 0), (1, 1), (0, 1))):
                for cc in range(NCH):
                    pr = sc2.tile([128, C], F16, tag="prod_x",
                                  name=f"tt_{m}_{cc}")
                    nc.vector.scalar_tensor_tensor(
                        out=pr[:],
                        in0=T_sb[:, C * (2 * cc + ca):C * (2 * cc + ca) + C],
                        scalar=1.0,
                        in1=T_sb[:, C * (2 * cc + cb):C * (2 * cc + cb) + C],
                        op0=OP.mult, op1=OP.mult,
                        accum_out=corr6[:, m * 2 + cc:m * 2 + cc + 1])
            cov6 = small.tile([128, 6], FP, tag="cov6")
            nc.vector.scalar_tensor_tensor(
                out=cov6[:], in0=corr6[:], scalar=-1.0 / B, in1=S6[:],
                op0=OP.mult, op1=OP.add)
            nc.vector.tensor_scalar(
                out=cov6[:], in0=cov6[:], scalar1=1.0 / NM1, scalar2=None,
                op0=OP.mult)
            _stk.close()  # release scratch + mean PSUM

            # ---- Phase C: closed-form 2x2 inverse sqrt, fold gamma
            def stile(tag):
                return small.tile([128, NCH], FP, tag=tag, name=tag)

            arr, cii, bri = stile("arr"), stile("cii"), stile("bri")
            nc.vector.tensor_scalar(out=arr[:], in0=cov6[:, 0:2],
                                    scalar1=EPS, scalar2=None, op0=OP.add)
            nc.vector.tensor_scalar(out=cii[:], in0=cov6[:, 2:4],
                                    scalar1=EPS, scalar2=None, op0=OP.add)
            nc.vector.tensor_copy(bri[:], cov6[:, 4:6])

            det, tmp = stile("det"), stile("tmp")
            nc.vector.tensor_tensor(out=det[:], in0=arr[:], in1=cii[:],
                                    op=OP.mult)
            nc.vector.tensor_tensor(out=tmp[:], in0=bri[:], in1=bri[:],
                                    op=OP.mult)
            nc.vector.tensor_tensor(out=det[:], in0=det[:], in1=tmp[:],
                                    op=OP.subtract)
            s_t = stile("s_t")
            nc.scalar.activation(s_t[:], det[:], AF.Sqrt)
            tsum = stile("tsum")
            nc.vector.tensor_tensor(out=tsum[:], in0=arr[:], in1=cii[:],
                                    op=OP.add)
            nc.vector.scalar_tensor_tensor(out=tsum[:], in0=s_t[:], scalar=2.0,
                                           in1=tsum[:], op0=OP.mult,
                                           op1=OP.add)
            tval = stile("tval")
            nc.scalar.activation(tval[:], tsum[:], AF.Sqrt)
            den, rden = stile("den"), stile("rden")
            nc.vector.tensor_tensor(out=den[:], in0=s_t[:], in1=tval[:],
                                    op=OP.mult)
            nc.vector.reciprocal(rden[:], den[:])

            w_rr, w_ii, wri = stile("w_rr"), stile("w_ii"), stile("wri")
            nc.vector.tensor_tensor(out=w_rr[:], in0=cii[:], in1=s_t[:],
                                    op=OP.add)
            nc.vector.tensor_tensor(out=w_rr[:], in0=w_rr[:], in1=rden[:],
                                    op=OP.mult)
            nc.vector.tensor_tensor(out=w_ii[:], in0=arr[:], in1=s_t[:],
                                    op=OP.add)
            nc.vector.tensor_tensor(out=w_ii[:], in0=w_ii[:], in1=rden[:],
                                    op=OP.mult)
            nc.vector.tensor_tensor(out=wri[:], in0=bri[:], in1=rden[:],
                                    op=OP.mult)
            nc.vector.tensor_scalar(out=wri[:], in0=wri[:], scalar1=-1.0,
                                    scalar2=None, op0=OP.mult)

            # A = G @ W
            a_rr, a_ri = stile("a_rr"), stile("a_ri")
            a_ir, a_ii = stile("a_ir"), stile("a_ii")
            u, v = stile("u"), stile("v")
            for dst, (wa, wb) in ((a_rr, (w_rr, wri)), (a_ri, (wri, w_ii))):
                nc.vector.tensor_tensor(out=u[:], in0=g_r_t[:], in1=wa[:],
                                        op=OP.mult)
                nc.vector.tensor_tensor(out=v[:], in0=g_i_t[:], in1=wb[:],
                                        op=OP.mult)
                nc.vector.tensor_tensor(out=dst[:], in0=u[:], in1=v[:],
                                        op=OP.subtract)
            for dst, (wa, wb) in ((a_ir, (w_rr, wri)), (a_ii, (wri, w_ii))):
                nc.vector.tensor_tensor(out=u[:], in0=g_i_t[:], in1=wa[:],
                                        op=OP.mult)
                nc.vector.tensor_tensor(out=v[:], in0=g_r_t[:], in1=wb[:],
                                        op=OP.mult)
                nc.vector.tensor_tensor(out=dst[:], in0=u[:], in1=v[:],
                                        op=OP.add)

            # ---- W tiles (fp16), diagonal per chunk: W[p, 2g+c]
            Ws = []
            for cc in range(NCH):
                W_r = small.tile([128, 256], F16, tag=f"W_r{cc}",
                                 name=f"W_r{cc}")
                W_i = small.tile([128, 256], F16, tag=f"W_i{cc}",
                                 name=f"W_i{cc}")
                for W, (ev, od) in ((W_r, (a_rr, a_ir)), (W_i, (a_ri, a_ii))):
                    Wv = W[:].rearrange("p (g c) -> p g c", c=2)
                    nc.vector.tensor_scalar(
                        out=Wv[:, :, 0], in0=ident_t[:],
                        scalar1=ev[:, cc:cc + 1], scalar2=None, op0=OP.mult)
                    nc.vector.tensor_scalar(
                        out=Wv[:, :, 1], in0=ident_t[:],
                        scalar1=od[:, cc:cc + 1], scalar2=None, op0=OP.mult)
                Ws.append((W_r, W_i))

            # ---- Arow rows 0..15: -A/MSCALE coefs in apply-column order,
            # via matmuls against the W tiles with even/odd selectors.
            from contextlib import ExitStack as _ES2
            _stk2 = _ES2()
            ps_t = _stk2.enter_context(
                tc.tile_pool(name="ps_t", bufs=1, space="PSUM"))
            psA = ps_t.tile([16, 2 * FSH], FP, tag="psA")
            for cc in range(NCH):
                W_r, W_i = Ws[cc]
                nc.tensor.matmul(psA[:, 256 * cc:256 * (cc + 1)],
                                 onesEv_t[:], W_r[:], start=True, stop=False)
                nc.tensor.matmul(psA[:, 256 * cc:256 * (cc + 1)],
                                 onesOd_t[:], W_i[:], start=False, stop=True)
            nc.vector.tensor_scalar(out=Arow[0:16, :], in0=psA[:],
                                    scalar1=-1.0 / MSCALE, scalar2=None,
                                    op0=OP.mult)

            # ---- Phase D: two-pass apply with interleaved emission
            stg = stgp.tile([128, NG * 512], F16, tag="stg")
            _stk3 = _ES2()
            ps_o = _stk3.enter_context(
                tc.tile_pool(name="ps_o", bufs=6, space="PSUM"))
            corrp = _stk3.enter_context(tc.tile_pool(name="corrp", bufs=2))

            def pass1(g):
                po = ps_o.tile([128, 512], FP, tag="po", name=f"po1_{g}")
                for cc in range(NCH):
                    W_r, W_i = Ws[cc]
                    nc.tensor.matmul(
                        po[:, 256 * cc:256 * (cc + 1)],
                        xsl(0, cc, 128 * g, 128), W_r[:],
                        start=True, stop=False)
                    nc.tensor.matmul(
                        po[:, 256 * cc:256 * (cc + 1)],
                        xsl(1, cc, 128 * g, 128), W_i[:],
                        start=False, stop=True)
                dst = stg[:, 512 * g:512 * (g + 1)]
                if g % 2 == 0:
                    nc.vector.tensor_copy(dst, po[:])
                else:
                    nc.scalar.copy(dst, po[:])

            def store4(g):
                g0 = g - 3
                dstd = out.rearrange("(a p) f -> p a f", p=128)[
                    :, g0:g0 + 4, :]
                srcd = stg[:, 512 * g0:512 * (g + 1)].rearrange(
                    "p (a q) -> p a q", q=512)
                if (g // 4) % 2 == 0:
                    nc.sync.dma_start(dstd, srcd)
                else:
                    nc.scalar.dma_start(dstd, srcd)

            def single(g):
                # one-pass group: K17 correction first (full region, start),
                # then the x-matmul sub-region accumulations (baseline's
                # beta-first PSUM pattern).
                po = ps_o.tile([128, 512], FP, tag="po", name=f"po1_{g}")
                nc.tensor.matmul(
                    po[:], mg[:, 128 * g:128 * (g + 1)], Arow[:],
                    start=True, stop=False)
                for cc in range(NCH):
                    W_r, W_i = Ws[cc]
                    nc.tensor.matmul(
                        po[:, 256 * cc:256 * (cc + 1)],
                        xsl(0, cc, 128 * g, 128), W_r[:],
                        start=False, stop=False)
                    nc.tensor.matmul(
                        po[:, 256 * cc:256 * (cc + 1)],
                        xsl(1, cc, 128 * g, 128), W_i[:],
                        start=False, stop=(cc == NCH - 1))
                dst = stg[:, 512 * g:512 * (g + 1)]
                if g % 2 == 0:
                    nc.vector.tensor_copy(dst, po[:])
                else:
                    nc.scalar.copy(dst, po[:])
                if g % 4 == 3:
                    store4(g)

            def pass2(g):
                po = ps_o.tile([128, 512], FP, tag="po", name=f"po2_{g}")
                nc.tensor.matmul(
                    po[:], mg[:, 128 * g:128 * (g + 1)], Arow[:],
                    start=True, stop=True)
                dst = stg[:, 512 * g:512 * (g + 1)]
                if g % 2 == 0:
                    nc.vector.scalar_tensor_tensor(
                        out=dst, in0=dst, scalar=1.0, in1=po[:],
                        op0=OP.mult, op1=OP.add)
                else:
                    ct = corrp.tile([128, 512], F16, tag="ct")
                    nc.scalar.copy(ct[:], po[:])
                    nc.gpsimd.tensor_tensor(
                        out=dst, in0=dst, in1=ct[:], op=OP.add)
                if g % 4 == 3:
                    store4(g)

            SPLIT = 44
            for g in range(SPLIT):
                pass1(g)
            for g in range(SPLIT, NG):
                single(g)
            for g in range(SPLIT):
                pass2(g)
            _stk3.close()
            _stk2.close()

    split_multi_waits(nc)
    return nc


_CACHE = {}


def _get_nc():
    if "nc" not in _CACHE:
        _CACHE["nc"] = build_bass()
    return _CACHE["nc"]


def _constants():
    if "consts" not in _CACHE:
        _CACHE["consts"] = {
            "ident": np.eye(128, dtype=np.float32),
            "onesF": np.full((128, 1), MSCALE / F, dtype=np.float16),
            "onesrow": np.ones((1, BC), dtype=ml_dtypes.float8_e4m3),
            "onesEv": np.tile((np.arange(16) % 2 == 0).astype(np.float16),
                              (128, 1)),
            "onesOd": np.tile((np.arange(16) % 2 == 1).astype(np.float16),
                              (128, 1)),
        }
    return _CACHE["consts"]


def _host_xt(xr, xi, fsl):
    """Build xt[p, 16384*comp + 8192*cc + j] = x_comp[j, 128*cc + p]."""
    halves = []
    for x in (xr, xi):
        xs = x[:, fsl].reshape(BC, NCH, 128)        # (j, cc, p)
        halves.append(np.transpose(xs, (2, 1, 0)).reshape(128, NCH * BC))
    return np.ascontiguousarray(
        np.concatenate(halves, axis=1)).astype(np.float16)


def kernel(x_real, x_imag, gamma_r, gamma_i, beta_r, beta_i):
    x_real = np.asarray(x_real, dtype=np.float32).reshape(BC, F)
    x_imag = np.asarray(x_imag, dtype=np.float32).reshape(BC, F)
    gamma_r = np.asarray(gamma_r, dtype=np.float32)
    gamma_i = np.asarray(gamma_i, dtype=np.float32)
    beta_r = np.asarray(beta_r, dtype=np.float32)
    beta_i = np.asarray(beta_i, dtype=np.float32)

    nc = _get_nc()
    consts = _constants()

    in_maps = []
    for k in range(NCORES):
        fsl = slice(FSH * k, FSH * (k + 1))
        g_r_t = np.ascontiguousarray(gamma_r[fsl].reshape(NCH, 128).T)
        g_i_t = np.ascontiguousarray(gamma_i[fsl].reshape(NCH, 128).T)
        beta_row = np.ascontiguousarray(
            np.stack([beta_r[fsl], beta_i[fsl]], axis=-1).reshape(1, 2 * FSH)
        ).astype(ml_dtypes.float8_e4m3)
        in_maps.append({
            "xt": _host_xt(x_real, x_imag, fsl),
            "g_r": g_r_t, "g_i": g_i_t, "beta_row": beta_row,
            **consts,
        })

    res = run_bass_kernel_spmd(nc, in_maps, list(range(NCORES)))

    full = np.empty((B, C, F, 2), dtype=np.float32)
    for k in range(NCORES):
        full[:, :, FSH * k:FSH * (k + 1)] = (
            np.asarray(res.results[k]["out"]).astype(np.float32)
            .reshape(B, C, FSH, 2)
        )
    return full


# revision 34
# speedup vs baseline: 1.5919x; 1.0640x over previous
"""ComplexLayerNorm Trainium2 kernel (8 NeuronCores, SPMD, F-sharded).

Math (see reference): per-feature 2x2 covariance whitening of (re, im) over
all B*C samples (centered with the batch-only mean mu_b), after subtracting
the complex mean over F, plus complex affine.

v4 design:
  * F-sharding: each core owns 256 features (2 chunks of 128 on partitions)
    and ALL B*C = 8192 samples; the covariance path is fully core-local.
  * Only the per-sample complex mean over F crosses cores: partials are
    scaled x64, quantized to fp8e4 (plenty for a mean-subtraction term) and
    ride ONE AllGather that overlaps the apply phase.  The 8-way shard sum
    AND the beta add are folded into a K=17 correction matmul (16 gathered
    mean rows + a ones row), whose rhs carries -A/64 coefficients.
  * Engine assignment respects the cost model: ACT does Square+accum and
    the PSUM mean-staging copies; Pool does the cross products and the
    first T-tree fold (TensorTensor); DVE does the x_i^2 products (2x
    fp16 TT), all second-moment accumulations (4x fp16 TensorScalarPtr
    accum), and the deep T-tree folds.
  * Two-pass apply: pass 1 (collective-independent) does the diagonal-W
    x-matmuls into PSUM and stages uncorrected A@x; pass 2 adds the K=17
    correction (DVE stt from PSUM on even groups, ACT-copy + Pool add on
    odd groups) and stores.  Emission interleaves the passes so the PE
    never idles waiting for the collective.
"""

import numpy as np
import ml_dtypes

import bass_rust
import concourse.bass as bass
import concourse.mybir as mybir
from concourse import tile
from concourse.bass_utils import run_bass_kernel_spmd


def split_multi_waits(nc):
    """The walrus build in this container allows only ONE sync-wait command
    per instruction; Tile emits several.  Split extras into preceding
    single-wait NoOps on the same engine (sequential waits == AND)."""
    cnt = 0
    for bb in nc.main_func.blocks:
        il = bb.instructions
        newlist = []
        changed = False
        for inst in list(il):
            si = inst.sync_info
            waits = list(si.on_wait) if si else []
            if len(waits) > 1:
                changed = True
                for w in waits[:-1]:
                    cnt += 1
                    nop = bass_rust.InstNoOp(name=f"I-wsplit-{cnt}")
                    nop.engine = inst.engine
                    nop.sync_info = mybir.SyncInfo(on_wait=[w], on_update=[])
                    newlist.append(nop)
                inst.sync_info = mybir.SyncInfo(
                    on_wait=[waits[-1]], on_update=list(si.on_update))
            newlist.append(inst)
        if changed:
            il[:] = newlist
    return cnt

FP = mybir.dt.float32
FR = mybir.dt.float32r
F16 = mybir.dt.float16
F8 = mybir.dt.float8e4
AF = mybir.ActivationFunctionType
OP = mybir.AluOpType
AX = mybir.AxisListType

B, C, F = 64, 128, 2048
NCORES = 8
FSH = F // NCORES           # 256 features per core
NCH = FSH // 128            # 2 f-chunks of 128 (on partitions)
BC = B * C                  # 8192 samples per core (full batch)
NBB = 4                     # bigblocks of 2048 samples for DMA streaming
BBS = BC // NBB             # 2048
NG = BC // 128              # 64 apply groups (128 samples each)
EPS = 1e-4
NM1 = float(B * C - 1)      # 8191
MSCALE = 64.0               # fp8 mean-partial scaling (folded into consts)


def build_bass():
    nc = bass.Bass()

    # x, fp16, f-on-partitions: xt[p, 16384*comp + 8192*cc + j]
    #   = x_comp[sample j, f_local = 128*cc + p]
    xt = nc.dram_tensor("xt", [128, 2 * NCH * BC], F16, kind="ExternalInput")
    ident = nc.dram_tensor("ident", [128, 128], FP, kind="ExternalInput")
    # mean-matmul weights: MSCALE/F
    onesF = nc.dram_tensor("onesF", [128, 1], F16, kind="ExternalInput")
    # even/odd row selectors for the Arow build
    onesEv = nc.dram_tensor("onesEv", [128, 16], F16, kind="ExternalInput")
    onesOd = nc.dram_tensor("onesOd", [128, 16], F16, kind="ExternalInput")
    # gamma for this core's shard, f-on-partitions: [128, NCH]
    g_r = nc.dram_tensor("g_r", [128, NCH], FP, kind="ExternalInput")
    g_i = nc.dram_tensor("g_i", [128, NCH], FP, kind="ExternalInput")
    # beta for this shard (fp8), apply-column order: [0, 256*cc + 2*g + c]
    beta_row = nc.dram_tensor("beta_row", [1, 2 * FSH], F8,
                              kind="ExternalInput")
    onesrow = nc.dram_tensor("onesrow", [1, BC], F8, kind="ExternalInput")

    out = nc.dram_tensor("out", [BC, 2 * FSH], F16, kind="ExternalOutput")

    with tile.TileContext(nc) as tc:
        with (
            tc.tile_pool(name="big", bufs=1) as big,
            tc.tile_pool(name="small", bufs=1) as small,
            tc.tile_pool(name="stg", bufs=1) as stgp,
            tc.tile_pool(name="dram", bufs=1, space="DRAM") as dram,
        ):
            # ---- constants (DMAs issued after the first x block; onesF
            # rides the scalar queue since the mean matmuls need it early)
            ident_t = small.tile([128, 128], FP, tag="ident")
            onesF_t = small.tile([128, 1], F16, tag="onesF")
            onesEv_t = small.tile([128, 16], F16, tag="onesEv")
            onesOd_t = small.tile([128, 16], F16, tag="onesOd")
            g_r_t = small.tile([128, NCH], FP, tag="g_r")
            g_i_t = small.tile([128, NCH], FP, tag="g_i")

            def emit_const_dmas():
                nc.scalar.dma_start(onesF_t[:], onesF[:])
                nc.scalar.dma_start(ident_t[:], ident[:])
                nc.scalar.dma_start(onesEv_t[:], onesEv[:])
                nc.scalar.dma_start(onesOd_t[:], onesOd[:])
                nc.scalar.dma_start(g_r_t[:], g_r[:])
                nc.scalar.dma_start(g_i_t[:], g_i[:])

            # mg: 16 gathered fp8 mean-partial rows + ones row (for beta)
            mg = small.tile([17, BC], F8, tag="mg")
            # Arow17: correction-matmul rhs (fp8); row 16 = beta
            Arow = small.tile([17, 2 * FSH], F8, tag="Arow")

            # ---- persistent x (fp16, f-on-partitions)
            xT = big.tile([128, 2 * NCH * BC], F16, tag="xT")

            # batch-sums over b: T_sb[p, 128*(2*cc+comp) + c]
            T_sb = small.tile([128, 2 * NCH * C], F16, tag="T_sb")
            # second-moment accumulator columns: 8 cols (cc,b) per
            # moment m (0=rr, 1=ii, 2=ri)
            S_acc = small.tile([128, 24], FP, tag="S_acc")

            from contextlib import ExitStack
            _stk = ExitStack()
            scr = _stk.enter_context(tc.tile_pool(name="scr", bufs=1))
            sc2 = _stk.enter_context(tc.tile_pool(name="sc2", bufs=2))
            trp = _stk.enter_context(tc.tile_pool(name="trp", bufs=2))
            msp = _stk.enter_context(tc.tile_pool(name="msp", bufs=2))
            ps_mean = _stk.enter_context(
                tc.tile_pool(name="ps_mean", bufs=2, space="PSUM"))
            ar_in = dram.tile([2, BC], F8, tag="ar_in")
            ar_out = dram.tile([16, BC], F8, tag="ar_out")

            def xsl(comp, cc, lo, n):
                return xT[:, 16384 * comp + BC * cc + lo:
                          16384 * comp + BC * cc + lo + n]

            def xdr(comp, cc, lo, n):
                return xt[:, 16384 * comp + BC * cc + lo:
                          16384 * comp + BC * cc + lo + n]

            # ---- Phase A: stream bigblocks; stats overlap the DMA
            prod_i = {}
            for b in range(NBB):
                lo = BBS * b
                h2, hb = b // 2, b % 2
                for cc in range(NCH):
                    for comp in range(2):
                        nc.sync.dma_start(xsl(comp, cc, lo, BBS),
                                          xdr(comp, cc, lo, BBS))
                    if b == 0 and cc == 0:
                        emit_const_dmas()
                    xr = xsl(0, cc, lo, BBS)
                    xi = xsl(1, cc, lo, BBS)
                    # S_rr on ACT: Square w/ accumulate (2048 grain)
                    sa = scr.tile([128, BBS], F16, tag="sq_act")
                    nc.scalar.activation(
                        sa[:], xr, AF.Square,
                        accum_out=S_acc[:, cc * 4 + b:cc * 4 + b + 1])
                    # x_i^2 products on DVE (2x fp16 TT) + 4x TSP accum
                    pi = scr.tile([128, BBS], F16, tag="prod_i",
                                  name=f"prod_i_{cc}_{b}")
                    nc.vector.tensor_tensor(out=pi[:], in0=xi, in1=xi,
                                            op=OP.mult)
                    nc.vector.tensor_scalar(
                        out=pi[:], in0=pi[:], scalar1=1.0, scalar2=0.0,
                        op0=OP.mult, op1=OP.add,
                        accum_out=S_acc[:, 8 + cc * 4 + b:
                                        8 + cc * 4 + b + 1])
                    # cross products on Pool, accum on DVE (4x TSP)
                    px = sc2.tile([128, BBS], F16, tag="prod_x")
                    nc.gpsimd.tensor_tensor(out=px[:], in0=xr, in1=xi,
                                            op=OP.mult)
                    nc.vector.tensor_scalar(
                        out=px[:], in0=px[:], scalar1=1.0, scalar2=0.0,
                        op0=OP.mult, op1=OP.add,
                        accum_out=S_acc[:, 16 + cc * 4 + b:
                                        16 + cc * 4 + b + 1])
                    # mean-partial matmuls: 512-block m of this bigblock
                    # lands on psum partition 64*(m%2), col 512*(m//2).
                    for comp in range(2):
                        if cc == 0 and comp == 0:
                            pm = [ps_mean.tile([128, 1024], FP,
                                               tag=f"psm{c2}",
                                               name=f"psm{c2}_{b}")
                                  for c2 in range(2)]
                            prod_i[("pm", b)] = pm
                        pm = prod_i[("pm", b)]
                        for m in range(4):
                            nc.tensor.matmul(
                                pm[comp][64 * (m % 2):64 * (m % 2) + 1,
                                         512 * (m // 2):512 * (m // 2) + 512],
                                onesF_t[:],
                                xsl(comp, cc, lo + 512 * m, 512),
                                start=(cc == 0), stop=(cc == NCH - 1),
                            )
                # T-tree at 2048 grain: fold1 on Pool, rest on DVE
                for cc in range(NCH):
                    for comp in range(2):
                        t1 = trp.tile([128, BBS // 2], F16, tag="t1")
                        nc.gpsimd.tensor_tensor(
                            out=t1[:], in0=xsl(comp, cc, lo, BBS // 2),
                            in1=xsl(comp, cc, lo + BBS // 2, BBS // 2),
                            op=OP.add)
                        t2 = trp.tile([128, BBS // 4], F16, tag="t2")
                        nc.vector.tensor_tensor(
                            out=t2[:], in0=t1[:, 0:BBS // 4],
                            in1=t1[:, BBS // 4:BBS // 2], op=OP.add)
                        t3 = trp.tile([128, BBS // 8], F16, tag="t3")
                        nc.vector.tensor_tensor(
                            out=t3[:], in0=t2[:, 0:BBS // 8],
                            in1=t2[:, BBS // 8:BBS // 4], op=OP.add)
                        td = T_sb[:, C * (2 * cc + comp):
                                  C * (2 * cc + comp) + C]
                        if b == 0:
                            nc.vector.tensor_tensor(
                                out=td, in0=t3[:, 0:C], in1=t3[:, C:2 * C],
                                op=OP.add)
                        else:
                            t4 = trp.tile([128, C], F16, tag="t4")
                            nc.vector.tensor_tensor(
                                out=t4[:], in0=t3[:, 0:C], in1=t3[:, C:2 * C],
                                op=OP.add)
                            nc.vector.tensor_tensor(
                                out=td, in0=td, in1=t4[:], op=OP.add)
                # stage mean partials PSUM -> SBUF (fp8) and drain to DRAM:
                # ar_in[comp, 2048*b + 1024*h + 512*m' + j] where data rows
                # sit at psum partitions {0,64} (m'), cols 512*h.
                pm = prod_i[("pm", b)]
                for comp in range(2):
                    mt = msp.tile([128, 1024], F8, tag=f"ms{comp}",
                                  name=f"ms{comp}_{b}")
                    nc.scalar.copy(mt[:], pm[comp][:])
                    src = mt[:].rearrange(
                        "(m o) (h j) -> m o h j", o=64, h=2)[:, 0:1]
                    dst = ar_in[comp:comp + 1,
                                BBS * b:BBS * (b + 1)].rearrange(
                        "o (h m j) -> m o h j", h=2, m=2)
                    nc.sync.dma_start(dst, src)

            # ---- AllGather the fp8 mean partials (overlaps everything below)
            nc.gpsimd.collective_compute(
                "AllGather", OP.bypass,
                replica_groups=[list(range(NCORES))],
                ins=[ar_in.opt()],
                outs=[ar_out.opt()],
            )
            # constants for pass 2 (DMA during the collective window)
            nc.scalar.dma_start(mg[16:17, :], onesrow[:])
            nc.scalar.dma_start(Arow[16:17, :], beta_row[:])
            # gather result in two halves so early groups start sooner
            for hf in range(2):
                nc.sync.dma_start(
                    mg[0:16, 4096 * hf:4096 * (hf + 1)],
                    ar_out[:, 4096 * hf:4096 * (hf + 1)])

            # ---- stats finalize: S6 (cols m*2+cc), corr6, cov6
            S6 = small.tile([128, 6], FP, tag="S6")
            nc.vector.tensor_reduce(
                S6[:, 0:2], S_acc[:, 0:8].rearrange("p (g b) -> p g b", b=4),
                AX.X, OP.add)
            nc.vector.tensor_reduce(
                S6[:, 2:6], S_acc[:, 8:24].rearrange("p (g b) -> p g b", b=4),
                AX.X, OP.add)
            corr6 = small.tile([128, 6], FP, tag="corr6")
            for m, (ca, cb) in enumerate(((0, 0), (1, 1), (0, 1))):
                for cc in range(NCH):
                    pr = sc2.tile([128, C], F16, tag="prod_x",
                                  name=f"tt_{m}_{cc}")
                    nc.vector.scalar_tensor_tensor(
                        out=pr[:],
                        in0=T_sb[:, C * (2 * cc + ca):C * (2 * cc + ca) + C],
                        scalar=1.0,
                        in1=T_sb[:, C * (2 * cc + cb):C * (2 * cc + cb) + C],
                        op0=OP.mult, op1=OP.mult,
                        accum_out=corr6[:, m * 2 + cc:m * 2 + cc + 1])
            cov6 = small.tile([128, 6], FP, tag="cov6")
            nc.vector.scalar_tensor_tensor(
                out=cov6[:], in0=corr6[:], scalar=-1.0 / B, in1=S6[:],
                op0=OP.mult, op1=OP.add)
            nc.vector.tensor_scalar(
                out=cov6[:], in0=cov6[:], scalar1=1.0 / NM1, scalar2=None,
                op0=OP.mult)
            _stk.close()  # release scratch + mean PSUM

            # ---- Phase C: closed-form 2x2 inverse sqrt, fold gamma
            def stile(tag):
                return small.tile([128, NCH], FP, tag=tag, name=tag)

            arr, cii = stile("arr"), stile("cii")
            bri = cov6[:, 4:6]
            nc.vector.tensor_scalar(out=arr[:], in0=cov6[:, 0:2],
                                    scalar1=EPS, scalar2=None, op0=OP.add)
            nc.vector.tensor_scalar(out=cii[:], in0=cov6[:, 2:4],
                                    scalar1=EPS, scalar2=None, op0=OP.add)

            det, tmp = stile("det"), stile("tmp")
            tsum0 = stile("tsum0")
            nc.vector.tensor_tensor(out=tmp[:], in0=bri, in1=bri,
                                    op=OP.mult)
            nc.vector.tensor_tensor(out=det[:], in0=arr[:], in1=cii[:],
                                    op=OP.mult)
            nc.vector.tensor_tensor(out=tsum0[:], in0=arr[:], in1=cii[:],
                                    op=OP.add)
            nc.vector.tensor_tensor(out=det[:], in0=det[:], in1=tmp[:],
                                    op=OP.subtract)
            s_t = stile("s_t")
            nc.scalar.activation(s_t[:], det[:], AF.Sqrt)
            tsum = stile("tsum")
            nc.vector.scalar_tensor_tensor(out=tsum[:], in0=s_t[:], scalar=2.0,
                                           in1=tsum0[:], op0=OP.mult,
                                           op1=OP.add)
            tval = stile("tval")
            nc.scalar.activation(tval[:], tsum[:], AF.Sqrt)
            den, rden = stile("den"), stile("rden")
            nc.vector.tensor_tensor(out=den[:], in0=s_t[:], in1=tval[:],
                                    op=OP.mult)
            nc.vector.reciprocal(rden[:], den[:])

            w_rr, w_ii, wri = stile("w_rr"), stile("w_ii"), stile("wri")
            nc.vector.tensor_tensor(out=w_rr[:], in0=cii[:], in1=s_t[:],
                                    op=OP.add)  # runs parallel to tval chain
            nc.vector.tensor_tensor(out=w_rr[:], in0=w_rr[:], in1=rden[:],
                                    op=OP.mult)
            nc.vector.tensor_tensor(out=w_ii[:], in0=arr[:], in1=s_t[:],
                                    op=OP.add)
            nc.vector.tensor_tensor(out=w_ii[:], in0=w_ii[:], in1=rden[:],
                                    op=OP.mult)
            nc.vector.scalar_tensor_tensor(out=wri[:], in0=bri,
                                           scalar=-1.0, in1=rden[:],
                                           op0=OP.mult, op1=OP.mult)

            # A = G @ W
            a_rr, a_ri = stile("a_rr"), stile("a_ri")
            a_ir, a_ii = stile("a_ir"), stile("a_ii")
            u, v = stile("u"), stile("v")
            for dst, (wa, wb) in ((a_rr, (w_rr, wri)), (a_ri, (wri, w_ii))):
                nc.vector.tensor_tensor(out=u[:], in0=g_r_t[:], in1=wa[:],
                                        op=OP.mult)
                nc.vector.tensor_tensor(out=v[:], in0=g_i_t[:], in1=wb[:],
                                        op=OP.mult)
                nc.vector.tensor_tensor(out=dst[:], in0=u[:], in1=v[:],
                                        op=OP.subtract)
            for dst, (wa, wb) in ((a_ir, (w_rr, wri)), (a_ii, (wri, w_ii))):
                nc.vector.tensor_tensor(out=u[:], in0=g_i_t[:], in1=wa[:],
                                        op=OP.mult)
                nc.vector.tensor_tensor(out=v[:], in0=g_r_t[:], in1=wb[:],
                                        op=OP.mult)
                nc.vector.tensor_tensor(out=dst[:], in0=u[:], in1=v[:],
                                        op=OP.add)

            # ---- W tiles (fp16), diagonal per chunk: W[p, 2g+c]
            Ws = []
            for cc in range(NCH):
                W_r = small.tile([128, 256], F16, tag=f"W_r{cc}",
                                 name=f"W_r{cc}")
                W_i = small.tile([128, 256], F16, tag=f"W_i{cc}",
                                 name=f"W_i{cc}")
                for W, (ev, od) in ((W_r, (a_rr, a_ir)), (W_i, (a_ri, a_ii))):
                    Wv = W[:].rearrange("p (g c) -> p g c", c=2)
                    nc.vector.tensor_scalar(
                        out=Wv[:, :, 0], in0=ident_t[:],
                        scalar1=ev[:, cc:cc + 1], scalar2=None, op0=OP.mult)
                    nc.vector.tensor_scalar(
                        out=Wv[:, :, 1], in0=ident_t[:],
                        scalar1=od[:, cc:cc + 1], scalar2=None, op0=OP.mult)
                Ws.append((W_r, W_i))

            # ---- Arow rows 0..15: -A/MSCALE coefs in apply-column order,
            # via matmuls against the W tiles with even/odd selectors.
            from contextlib import ExitStack as _ES2
            _stk2 = _ES2()
            ps_t = _stk2.enter_context(
                tc.tile_pool(name="ps_t", bufs=1, space="PSUM"))
            psA = ps_t.tile([16, 2 * FSH], FP, tag="psA")
            for cc in range(NCH):
                W_r, W_i = Ws[cc]
                nc.tensor.matmul(psA[:, 256 * cc:256 * (cc + 1)],
                                 onesEv_t[:], W_r[:], start=True, stop=False)
                nc.tensor.matmul(psA[:, 256 * cc:256 * (cc + 1)],
                                 onesOd_t[:], W_i[:], start=False, stop=True)
            nc.vector.tensor_scalar(out=Arow[0:16, :], in0=psA[:],
                                    scalar1=-1.0 / MSCALE, scalar2=None,
                                    op0=OP.mult)
            _stk2.close()

            # ---- Phase D: two-pass apply with interleaved emission
            stg = stgp.tile([128, NG * 512], F16, tag="stg")
            _stk3 = _ES2()
            ps_o = _stk3.enter_context(
                tc.tile_pool(name="ps_o", bufs=7, space="PSUM"))
            corrp = _stk3.enter_context(tc.tile_pool(name="corrp", bufs=2))

            def pass1(g):
                po = ps_o.tile([128, 512], FP, tag="po", name=f"po1_{g}")
                for cc in range(NCH):
                    W_r, W_i = Ws[cc]
                    nc.tensor.matmul(
                        po[:, 256 * cc:256 * (cc + 1)],
                        xsl(0, cc, 128 * g, 128), W_r[:],
                        start=True, stop=False)
                    nc.tensor.matmul(
                        po[:, 256 * cc:256 * (cc + 1)],
                        xsl(1, cc, 128 * g, 128), W_i[:],
                        start=False, stop=True)
                dst = stg[:, 512 * g:512 * (g + 1)]
                if g % 2 == 0:
                    nc.vector.tensor_copy(dst, po[:])
                else:
                    nc.scalar.copy(dst, po[:])

            def store4(g):
                g0 = g - 3
                dstd = out.rearrange("(a p) f -> p a f", p=128)[
                    :, g0:g0 + 4, :]
                srcd = stg[:, 512 * g0:512 * (g + 1)].rearrange(
                    "p (a q) -> p a q", q=512)
                if (g // 4) % 2 == 0:
                    nc.sync.dma_start(dstd, srcd)
                else:
                    nc.scalar.dma_start(dstd, srcd)

            def single(g):
                # one-pass group: K17 correction first (full region, start),
                # then the x-matmul sub-region accumulations (baseline's
                # beta-first PSUM pattern).
                po = ps_o.tile([128, 512], FP, tag="po", name=f"po1_{g}")
                nc.tensor.matmul(
                    po[:], mg[:, 128 * g:128 * (g + 1)], Arow[:],
                    start=True, stop=False)
                for cc in range(NCH):
                    W_r, W_i = Ws[cc]
                    nc.tensor.matmul(
                        po[:, 256 * cc:256 * (cc + 1)],
                        xsl(0, cc, 128 * g, 128), W_r[:],
                        start=False, stop=False)
                    nc.tensor.matmul(
                        po[:, 256 * cc:256 * (cc + 1)],
                        xsl(1, cc, 128 * g, 128), W_i[:],
                        start=False, stop=(cc == NCH - 1))
                dst = stg[:, 512 * g:512 * (g + 1)]
                if g % 2 == 0:
                    nc.vector.tensor_copy(dst, po[:])
                else:
                    nc.scalar.copy(dst, po[:])
                if g % 4 == 3:
                    store4(g)

            def pass2(g):
                po = ps_o.tile([128, 512], FP, tag="po", name=f"po2_{g}")
                nc.tensor.matmul(
                    po[:], mg[:, 128 * g:128 * (g + 1)], Arow[:],
                    start=True, stop=True)
                dst = stg[:, 512 * g:512 * (g + 1)]
                if g % 2 == 0:
                    nc.vector.scalar_tensor_tensor(
                        out=dst, in0=dst, scalar=1.0, in1=po[:],
                        op0=OP.mult, op1=OP.add)
                else:
                    ct = corrp.tile([128, 512], F16, tag="ct")
                    nc.scalar.copy(ct[:], po[:])
                    nc.gpsimd.tensor_tensor(
                        out=dst, in0=dst, in1=ct[:], op=OP.add)
                if g % 4 == 3:
                    store4(g)

            SPLIT = 40
            for g in range(SPLIT):
                pass1(g)
            p2 = 0
            for i, g in enumerate(range(SPLIT, NG)):
                single(g)
                want = (i + 1) * SPLIT // (NG - SPLIT)
                while p2 < min(want, SPLIT):
                    pass2(p2)
                    p2 += 1
            while p2 < SPLIT:
                pass2(p2)
                p2 += 1
            _stk3.close()

    split_multi_waits(nc)
    return nc


_CACHE = {}


def _get_nc():
    if "nc" not in _CACHE:
        _CACHE["nc"] = build_bass()
    return _CACHE["nc"]


def _constants():
    if "consts" not in _CACHE:
        _CACHE["consts"] = {
            "ident": np.eye(128, dtype=np.float32),
            "onesF": np.full((128, 1), MSCALE / F, dtype=np.float16),
            "onesrow": np.ones((1, BC), dtype=ml_dtypes.float8_e4m3),
            "onesEv": np.tile((np.arange(16) % 2 == 0).astype(np.float16),
                              (128, 1)),
            "onesOd": np.tile((np.arange(16) % 2 == 1).astype(np.float16),
                              (128, 1)),
        }
    return _CACHE["consts"]


def _host_xt(xr, xi, fsl):
    """Build xt[p, 16384*comp + 8192*cc + j] = x_comp[j, 128*cc + p]."""
    halves = []
    for x in (xr, xi):
        xs = x[:, fsl].reshape(BC, NCH, 128)        # (j, cc, p)
        halves.append(np.transpose(xs, (2, 1, 0)).reshape(128, NCH * BC))
    return np.ascontiguousarray(
        np.concatenate(halves, axis=1)).astype(np.float16)


def kernel(x_real, x_imag, gamma_r, gamma_i, beta_r, beta_i):
    x_real = np.asarray(x_real, dtype=np.float32).reshape(BC, F)
    x_imag = np.asarray(x_imag, dtype=np.float32).reshape(BC, F)
    gamma_r = np.asarray(gamma_r, dtype=np.float32)
    gamma_i = np.asarray(gamma_i, dtype=np.float32)
    beta_r = np.asarray(beta_r, dtype=np.float32)
    beta_i = np.asarray(beta_i, dtype=np.float32)

    nc = _get_nc()
    consts = _constants()

    in_maps = []
    for k in range(NCORES):
        fsl = slice(FSH * k, FSH * (k + 1))
        g_r_t = np.ascontiguousarray(gamma_r[fsl].reshape(NCH, 128).T)
        g_i_t = np.ascontiguousarray(gamma_i[fsl].reshape(NCH, 128).T)
        beta_row = np.ascontiguousarray(
            np.stack([beta_r[fsl], beta_i[fsl]], axis=-1).reshape(1, 2 * FSH)
        ).astype(ml_dtypes.float8_e4m3)
        in_maps.append({
            "xt": _host_xt(x_real, x_imag, fsl),
            "g_r": g_r_t, "g_i": g_i_t, "beta_row": beta_row,
            **consts,
        })

    res = run_bass_kernel_spmd(nc, in_maps, list(range(NCORES)))

    full = np.empty((B, C, F, 2), dtype=np.float32)
    for k in range(NCORES):
        full[:, :, FSH * k:FSH * (k + 1)] = (
            np.asarray(res.results[k]["out"]).astype(np.float32)
            .reshape(B, C, FSH, 2)
        )
    return full
